# revision 2
# baseline (speedup 1.0000x reference)
"""BertAttention Trainium2 kernel v2 (8 NeuronCores, SPMD, no collectives).

Sharding: DP over batch (2) x sequence-parallel over 512-row query blocks (4).
All heavy matmuls run fp8e4 DoubleRow (2 contraction subtiles per instruction,
0.5 cyc/output-column):

  - host sends xT pre-transposed fp8 (sequence rolled so this core's query
    block is cols [0,512)), weights pre-tiled fp8 scaled by 32 (keeps
    W ~N(0,0.02^2) out of the fp8 subnormal range), biases f32.
  - attention_mask folded on host: em = exp(mask) scales the vaug rows
    (data cols and the denominator ones-col), so exp needs no bias;
    bv and bo fold into the residual (xq += bo + bv @ Wo).
  - Q/K proj per head-pair in [d, seq] orientation; evac adds 32*bias, casts
    fp8 (DVE). scores contract d=64 as two 32-partition DoubleRow subtiles
    via DMA-rearranged kdr/qdr. V computed in natural [sk, d] orientation,
    4 head-pairs per PSUM; evac scales by em[sk], casts fp8 into vaug
    [128, j, 96] tiles (col 64 = em, cols 65..95 zero pad - DoubleRow wants
    M % 32 == 0). ctx PSUM row 64 accumulates the softmax denominator.
  - softmax exp on Act engine; a subset of tiles is offloaded to DVE via a
    Schraudolph-style trick: u8 = round(8*log2(pt)+56) bitcast to fp8e4.
  - ctx /= den via fast reciprocal + ones(1.0) K=1 matmul broadcast; the
    fp8 cast multiplies by 32 (scalar_tensor_tensor) so ctxT = 1024*ctx.
  - out-proj evac: Act scale-copy 2^-15 + DVE residual add; LayerNorm via
    bn_stats/aggr, Act Rsqrt, fused DVE stt + Pool tensor_scalar/add tail.
"""

import numpy as np
import ml_dtypes

import bass_rust as _br
import concourse.bass as bass
import concourse.tile as tile
from concourse import mybir
from concourse.bass_utils import run_bass_kernel_spmd

F32 = mybir.dt.float32
F32R = mybir.dt.float32r
F8 = mybir.dt.float8e4
U8 = mybir.dt.uint8
DR = mybir.MatmulPerfMode.DoubleRow
ADD = mybir.AluOpType.add
MULT = mybir.AluOpType.mult
SUB = mybir.AluOpType.subtract

S = 2048
H = 1024
P = 128
SQ = 512          # query rows per core
NKT = S // P      # 16 sk tiles
HC = H // P       # 8 h-chunks
NPAIR = 8         # head pairs
WS = 32.0         # fp8 weight scale
EPS = 1e-12
EXP_SCALE = 0.125 / (WS * WS)     # 2^-13
OUT_SCALE = 1.0 / (WS * WS * WS)  # 2^-15
# Schraudolph-on-DVE offload: these sk-tiles' exps run on DVE as
# u8 = s * SCH_A + SCH_B bitcast fp8e4 (exp2 piecewise-linear approx).
# Per-pair sets sized to balance DVE load (pairs 0-3 also carry V evacs).
SCH_BY_PAIR = (
    (),
    (13,),
    (13,),
    (13,),
    (3, 7, 11, 14),
    (3, 7, 11, 14),
    (3, 7, 11, 14),
    (2, 5, 8, 11, 14),
)
SCH_A = 8 * 1.4426950408889634 * EXP_SCALE
SCH_B = 56.0
# V-projection emission schedule: group 0 just-in-time inside pair 0;
# group 1 spread over pairs 1-3 (list of t per (pair, t2 slot)).
V_SCHED = {
    1: {0: (0, 1), 1: (2, 3), 2: (4, 5)},
    2: {0: (6, 7), 1: (8, 9), 2: (10,)},
    3: {0: (11, 12), 1: (13, 14), 2: (15,)},
}

_wait_ctr = [0]


def _split_excess_waits(nc, limit=1):
    """walrus in this container rejects >1-2 sem waits on several opcode
    structs; move excess waits onto same-engine NoOps inserted just before."""
    for f in nc.m.functions:
        for bb in f.blocks:
            insts = bb.instructions
            out = []
            dirty = False
            for inst in insts:
                si = inst.sync_info
                waits = list(si.on_wait) if si and si.on_wait else []
                if len(waits) > limit and inst.engine != mybir.EngineType.Unassigned:
                    for i in range(0, len(waits) - limit, limit):
                        _wait_ctr[0] += 1
                        nop = _br.InstNoOp(
                            name=f"I-waitsplit-{_wait_ctr[0]}", ins=[], outs=[]
                        )
                        nop.engine = inst.engine
                        nop.sync_info = mybir.SyncInfo(
                            on_wait=waits[i : i + limit], on_update=[]
                        )
                        out.append(nop)
                    si.on_wait = waits[len(waits) - limit :]
                    dirty = True
                out.append(inst)
            if dirty:
                bb.instructions = out
    return nc


def _ap(t, off, dims):
    """Strided AP over a tile's partition range: dims = [[stride, count], ...]."""
    return bass.AP(
        tensor=t.tensor,
        offset=t.offset + off,
        ap=[list(t.ap[0])] + [list(d) for d in dims],
    )


# vaug layout per partition: [t2(8), j(2), m(8), h(2), c(96)]
VA_C = 96
VA_H = VA_C            # stride of h
VA_M = 2 * VA_C        # 192
VA_J = 8 * VA_M        # 1536
VA_T2 = 2 * VA_J       # 3072
VA_TOTAL = 8 * VA_T2   # 24576


def build_nc():
    nc = bass.Bass()

    xT_d = nc.dram_tensor("xT", [P, HC, S], F8, kind="ExternalInput")
    xq_d = nc.dram_tensor("xq", [SQ, H], F32, kind="ExternalInput")  # +bo+bv@Wo
    wq_d = nc.dram_tensor("wq", [NPAIR, P, HC * P], F8, kind="ExternalInput")
    wk_d = nc.dram_tensor("wk", [NPAIR, P, HC * P], F8, kind="ExternalInput")
    wv_d = nc.dram_tensor("wv", [2, P, HC * 512], F8, kind="ExternalInput")
    wo_d = nc.dram_tensor("wo", [P, HC * H], F8, kind="ExternalInput")
    bqk_d = nc.dram_tensor("bqk", [P, 16], F32, kind="ExternalInput")  # 32*(bq|bk)
    gamma_d = nc.dram_tensor("gamma", [H], F32, kind="ExternalInput")
    beta_d = nc.dram_tensor("beta", [H], F32, kind="ExternalInput")
    em_d = nc.dram_tensor("em", [P, NKT], F32, kind="ExternalInput")  # exp(mask)
    out_d = nc.dram_tensor("out", [SQ, H], F32, kind="ExternalOutput")

    with tile.TileContext(nc) as tc, nc.allow_low_precision(
        reason="fp8 DoubleRow matmuls; accumulation stays fp32 in PSUM"
    ):
        consts = tc.alloc_tile_pool(name="consts", bufs=1)
        xT_pool = tc.alloc_tile_pool(name="xT", bufs=1)
        va_pool = tc.alloc_tile_pool(name="va", bufs=1)
        wv_pool = tc.alloc_tile_pool(name="wv", bufs=1)
        ctxT_pool = tc.alloc_tile_pool(name="ctxT", bufs=1)
        xq_pool = tc.alloc_tile_pool(name="xq", bufs=1)
        w_pool = tc.alloc_tile_pool(name="w", bufs=3)
        kv_pool = tc.alloc_tile_pool(name="kv", bufs=4)
        pt_pool = tc.alloc_tile_pool(name="pt", bufs=6)
        r_pool = tc.alloc_tile_pool(name="r", bufs=4)
        ln_pool = tc.alloc_tile_pool(name="ln", bufs=2)
        ps_mm = tc.alloc_tile_pool(name="ps_mm", bufs=2, space="PSUM")
        ps_s = tc.alloc_tile_pool(name="ps_s", bufs=2, space="PSUM")
        ps_ctx = tc.alloc_tile_pool(name="ps_ctx", bufs=2, space="PSUM")

        # ---- critical-path DMAs first (HWDGE is ~620ns PER DMA op and the
        # transfers mostly serialize, so few, large, well-ordered DMAs
        # matter): xT query cols, then wq0/wk0 ----
        xT_all = xT_pool.tile([P, HC * S], F8, name="xT_all", tag="xT_all")
        xT_v = xT_all.rearrange("p (c s) -> p c s", c=HC)
        # query-block columns (0..512) of every chunk in one strided DMA:
        # unblocks the q projection and the first k block early
        nc.sync.dma_start(out=xT_v[:, :, 0:512], in_=xT_d[:, :, 0:512])
        wq0 = w_pool.tile([P, HC * P], F8, tag="wq_m")
        nc.sync.dma_start(out=wq0, in_=wq_d[0])
        wk0 = w_pool.tile([P, HC * P], F8, tag="wk_m")
        nc.sync.dma_start(out=wk0, in_=wk_d[0])
        bqk = consts.tile([P, 16], F32, tag="bqk")
        nc.sync.dma_start(out=bqk, in_=bqk_d[:, :])
        em = consts.tile([P, NKT], F32, tag="em")
        nc.sync.dma_start(out=em, in_=em_d[:, :])

        # ---- small consts / VA init (Pool) ----
        eps_t = consts.tile([P, 1], F32, tag="eps")
        nc.vector.memset(eps_t, EPS)
        ones32 = consts.tile([1, 64], F32, tag="ones32")
        nc.vector.memset(ones32, 1.0)
        ones32_r = ones32.bitcast(F32R)
        zeros16 = consts.tile([P, 16], F32, tag="zeros16")
        nc.gpsimd.memset(zeros16, 0.0)

        VA = va_pool.tile([P, VA_TOTAL], F8, name="VA", tag="VA")
        for t2 in range(8):
            for j in range(2):
                base = VA_T2 * t2 + VA_J * j
                # zero the pad block (cols 64..95)
                nc.gpsimd.memset(
                    _ap(VA, base + 64, [[VA_M, 8], [VA_H, 2], [1, 32]]), 0.0
                )
        for t in range(NKT):
            # denominator col (64) = em[:, t] per (m, h)
            base = VA_T2 * (t // 2) + VA_J * (t % 2) + 64
            nc.gpsimd.tensor_scalar(
                out=_ap(VA, base, [[VA_M, 8], [VA_H, 2]]),
                in0=_ap(zeros16, 0, [[2, 8], [1, 2]]),
                scalar1=em[:, t : t + 1],
                scalar2=None,
                op0=ADD,
            )

        ctxT_all = ctxT_pool.tile([P, NPAIR * SQ], F8, name="ctxT", tag="ctxT")

        # ---- AP helpers ----
        def xt_rhs(c2, s0, ns):
            return _ap(xT_all, 2 * c2 * S + s0, [[S, 2], [1, ns]])

        def xt_lhsT(c2, t):
            return _ap(xT_all, 2 * c2 * S + t * P, [[S, 2], [1, P]])

        def w_lhsT(w, c2):
            return _ap(w, 2 * c2 * P, [[P, 2], [1, P]])

        def wv_rhs(g, c2):
            return _ap(wv_g[g], 2 * c2 * 512, [[512, 2], [1, 512]])

        def va_lhsT(t2, m, h):
            return _ap(VA, VA_T2 * t2 + VA_M * m + VA_H * h, [[VA_J, 2], [1, VA_C]])

        def va_dst(t, g):
            return _ap(
                VA,
                VA_T2 * (t // 2) + VA_J * (t % 2) + VA_M * (4 * g),
                [[VA_M, 4], [VA_H, 2], [1, 64]],
            )

        def kdr_lhsT(kdr, h, t):
            return _ap(kdr, 4096 * h + t * P, [[2048, 2], [1, P]])

        def qdr_rhs(qdr, h):
            return _ap(qdr, 1024 * h, [[512, 2], [1, 512]])

        def pt_rhs(pt, h):
            return _ap(pt, 512 * h, [[1024, 2], [1, 512]])

        def ctxT_lhsT(c2, st):
            return _ap(ctxT_all, 2 * c2 * SQ + st * P, [[SQ, 2], [1, P]])

        # ---- per-pair Q/K projection + DoubleRow rearrangements ----
        def emit_w_dma(m):
            wq_m = w_pool.tile([P, HC * P], F8, tag="wq_m")
            nc.sync.dma_start(out=wq_m, in_=wq_d[m])
            wk_m = w_pool.tile([P, HC * P], F8, tag="wk_m")
            nc.sync.dma_start(out=wk_m, in_=wk_d[m])
            return wq_m, wk_m

        def emit_q(m, wq_m):
            ps = ps_mm.tile([P, 512], F32, name="ps", tag="ps")
            for c2 in range(4):
                nc.tensor.matmul(
                    ps,
                    w_lhsT(wq_m, c2),
                    xt_rhs(c2, 0, 512),
                    start=(c2 == 0),
                    stop=(c2 == 3),
                    perf_mode=DR,
                )
            q_sb = kv_pool.tile([P, 512], F8, tag="q_sb")
            nc.vector.tensor_scalar_add(q_sb, ps, bqk[:, m : m + 1])
            qdr = kv_pool.tile([32, 2048], F8, tag="qdr")
            for h in range(2):
                for j in range(2):
                    nc.sync.dma_start(
                        out=qdr[:, 1024 * h + 512 * j : 1024 * h + 512 * (j + 1)],
                        in_=q_sb[64 * h + 32 * j : 64 * h + 32 * j + 32, :],
                    )
            return qdr

        def emit_k_block(m, wk_m, k_sb, n):
            ps = ps_mm.tile([P, 512], F32, name="ps", tag="ps")
            for c2 in range(4):
                nc.tensor.matmul(
                    ps,
                    w_lhsT(wk_m, c2),
                    xt_rhs(c2, n * 512, 512),
                    start=(c2 == 0),
                    stop=(c2 == 3),
                    perf_mode=DR,
                )
            nc.vector.tensor_scalar_add(
                k_sb[:, n * 512 : (n + 1) * 512], ps, bqk[:, 8 + m : 9 + m]
            )

        def emit_kdr_dma(k_sb, kdr, c0, c1):
            for h in range(2):
                for j in range(2):
                    nc.sync.dma_start(
                        out=kdr[:, 4096 * h + 2048 * j + c0 : 4096 * h + 2048 * j + c1],
                        in_=k_sb[64 * h + 32 * j : 64 * h + 32 * j + 32, c0:c1],
                    )

        def emit_qk_proj(m, wq_m=None, wk_m=None):
            if wq_m is None:
                wq_m, wk_m = emit_w_dma(m)
            qdr = emit_q(m, wq_m)
            k_sb = kv_pool.tile([P, S], F8, tag="k_sb")
            kdr = kv_pool.tile([32, 8192], F8, tag="kdr")
            emit_k_block(m, wk_m, k_sb, 0)
            emit_kdr_dma(k_sb, kdr, 0, 512)
            # startup only (this path only runs for pair 0): wv group 0 must
            # beat the first ctx tiles, then the rest of xT for k blocks 1-3
            wvt0 = wv_pool.tile([P, HC * 512], F8, name="wv0", tag="wv0")
            nc.sync.dma_start(out=wvt0, in_=wv_d[0])
            wv_g.append(wvt0)
            nc.sync.dma_start(out=xT_v[:, :, 512:1280], in_=xT_d[:, :, 512:1280])
            nc.sync.dma_start(out=xT_v[:, :, 1280:S], in_=xT_d[:, :, 1280:S])
            for n in range(1, 4):
                emit_k_block(m, wk_m, k_sb, n)
            emit_kdr_dma(k_sb, kdr, 512, 2048)
            return qdr, kdr

        def emit_v(g, t):
            ps = ps_mm.tile([P, 512], F32, name="vps", tag="ps")
            for c2 in range(4):
                nc.tensor.matmul(
                    ps,
                    xt_lhsT(c2, t),
                    wv_rhs(g, c2),
                    start=(c2 == 0),
                    stop=(c2 == 3),
                    perf_mode=DR,
                )
            nc.vector.tensor_scalar(
                out=va_dst(t, g),
                in0=_ap(ps, 0, [[P, 4], [64, 2], [1, 64]]),
                scalar1=em[:, t : t + 1],
                scalar2=None,
                op0=MULT,
            )

        # ---- main loop ----
        def emit_norm(m, ctx_ps):
            # normalize + fp8 ctxT (= 1024 * ctx); recips first so the DVE
            # chain pipelines across both heads
            rrs = []
            for h in range(2):
                rr = r_pool.tile([1, 512], F32R, tag="rr")
                nc.vector.reciprocal(rr, ctx_ps[h][64:65, :])
                rrs.append(rr)
            bcs = []
            for h in range(2):
                bc_ps = ps_mm.tile([64, 512], F32, name="bc_ps", tag="ps")
                nc.tensor.matmul(bc_ps, ones32_r, rrs[h], start=True, stop=True)
                bcs.append(bc_ps)
            rbs = []
            for h in range(2):
                rb = r_pool.tile([64, 512], F32, tag="rb")
                nc.vector.tensor_copy(rb, bcs[h])
                rbs.append(rb)
            for h in range(2):
                nc.vector.scalar_tensor_tensor(
                    out=ctxT_all[64 * h : 64 * h + 64, m * SQ : (m + 1) * SQ],
                    in0=ctx_ps[h][0:64, :],
                    scalar=WS,
                    in1=rbs[h],
                    op0=MULT,
                    op1=MULT,
                )

        wv_g = []
        dr_cur = emit_qk_proj(0, wq0, wk0)
        wvt1 = wv_pool.tile([P, HC * 512], F8, name="wv1", tag="wv1")
        nc.sync.dma_start(out=wvt1, in_=wv_d[1])
        wv_g.append(wvt1)
        prev_norm = None  # (m, ctx_ps) awaiting normalization
        w_next = None
        k_next = None
        for m in range(NPAIR):
            qdr, kdr = dr_cur
            ctx_ps = [
                ps_ctx.tile([P, 512], F32, name=f"ctx{h}", tag="ctx_ps")
                for h in range(2)
            ]

            def emit_ctx(e_t2, e_pt):
                for h in range(2):
                    nc.tensor.matmul(
                        ctx_ps[h][0:96, :],
                        va_lhsT(e_t2, m, h),
                        pt_rhs(e_pt, h),
                        start=(e_t2 == 0),
                        stop=(e_t2 == 7),
                        perf_mode=DR,
                    )

            pend_ctx = []
            for t2 in range(8):
                pt = pt_pool.tile([P, 2048], F8, name="pt", tag="pt")
                for j in range(2):
                    t = 2 * t2 + j
                    s2 = ps_s.tile([P, 1024], F32, name="s2", tag="s2")
                    for h in range(2):
                        nc.tensor.matmul(
                            s2[:, 512 * h : 512 * (h + 1)],
                            kdr_lhsT(kdr, h, t),
                            qdr_rhs(qdr, h),
                            start=True,
                            stop=True,
                            perf_mode=DR,
                        )
                    if t in SCH_BY_PAIR[m]:
                        nc.vector.tensor_scalar(
                            out=pt[:, 1024 * j : 1024 * (j + 1)].bitcast(U8),
                            in0=s2,
                            scalar1=SCH_A,
                            scalar2=SCH_B,
                            op0=MULT,
                            op1=ADD,
                        )
                    else:
                        nc.scalar.activation(
                            pt[:, 1024 * j : 1024 * (j + 1)],
                            s2,
                            mybir.ActivationFunctionType.Exp,
                            scale=EXP_SCALE,
                        )
                # interleaved work while Act chews the exps; t2==2 keeps the
                # DVE norm chain away from the Schraudolph slots
                if t2 == 4 and prev_norm is not None:
                    emit_norm(*prev_norm)
                    prev_norm = None
                if m == 0:
                    emit_v(0, 2 * t2)
                    emit_v(0, 2 * t2 + 1)
                elif m in V_SCHED and t2 in V_SCHED[m]:
                    for tv in V_SCHED[m][t2]:
                        emit_v(1, tv)
                if m < NPAIR - 1:
                    # next pair's proj, spread across t2 slots
                    if t2 == 1:
                        w_next = emit_w_dma(m + 1)
                    elif t2 == 2:
                        q_next = emit_q(m + 1, w_next[0])
                        k_next = (
                            kv_pool.tile([P, S], F8, tag="k_sb", name="k_sb"),
                            kv_pool.tile([32, 8192], F8, tag="kdr", name="kdr"),
                        )
                        dr_next = (q_next, k_next[1])
                    elif t2 in (3, 4, 5, 6):
                        emit_k_block(m + 1, w_next[1], k_next[0], t2 - 3)
                        if t2 == 6:
                            emit_kdr_dma(k_next[0], k_next[1], 0, 2048)
                if t2 == 5 and 2 <= m <= 6:
                    # deferred non-critical input DMAs, spread one pair apart
                    # so they never clump ahead of the next pair's weights
                    if m == 2:
                        xq = []
                    if m <= 5:
                        xqt = xq_pool.tile(
                            [P, H], F32, name=f"xq{m-2}", tag=f"xq{m-2}"
                        )
                        nc.sync.dma_start(
                            out=xqt, in_=xq_d[(m - 2) * P : (m - 1) * P, :]
                        )
                        xq.append(xqt)
                    if m == 3:
                        gamma_bc = consts.tile([P, H], F32, tag="gamma_bc")
                        nc.sync.dma_start(
                            out=gamma_bc, in_=gamma_d[:].partition_broadcast(P)
                        )
                    elif m == 4:
                        beta_bc = consts.tile([P, H], F32, tag="beta_bc")
                        nc.sync.dma_start(
                            out=beta_bc, in_=beta_d[:].partition_broadcast(P)
                        )
                    elif m == 6:
                        wo_sb = wv_pool.tile([P, HC * H], F8, name="wo", tag="wo")
                        nc.sync.dma_start(out=wo_sb, in_=wo_d[:, :])
                # defer ctx emission by 2 slots so a pending WAR on the
                # ctx accumulators (prev pair's norm) can't head-of-line
                # block the scores stream on the in-order PE
                pend_ctx.append((t2, pt))
                if len(pend_ctx) > 2:
                    emit_ctx(*pend_ctx.pop(0))
            for e in pend_ctx:
                emit_ctx(*e)
            prev_norm = (m, ctx_ps)
            dr_cur = dr_next
        emit_norm(*prev_norm)

        def wo_rhs(c2, nch):
            return _ap(wo_sb, 2 * c2 * H + nch * 512, [[H, 2], [1, 512]])

        # ---- output projection + residual + LayerNorm ----
        for st in range(SQ // P):
            h_sb = ln_pool.tile([P, H], F32, tag="h_sb")
            for nch in range(2):
                ps = ps_mm.tile([P, 512], F32, name="ops", tag="ps")
                for c2 in range(4):
                    nc.tensor.matmul(
                        ps,
                        ctxT_lhsT(c2, st),
                        wo_rhs(c2, nch),
                        start=(c2 == 0),
                        stop=(c2 == 3),
                        perf_mode=DR,
                    )
                h0 = ln_pool.tile([P, 512], F32, tag="h0")
                nc.scalar.mul(h0, ps, OUT_SCALE)
                nc.vector.tensor_add(
                    h_sb[:, nch * 512 : (nch + 1) * 512],
                    h0,
                    xq[st][:, nch * 512 : (nch + 1) * 512],
                )
            stats = ln_pool.tile([P, 2, 6], F32, tag="stats")
            for gg in range(2):
                nc.vector.bn_stats(
                    out=stats[:, gg, :], in_=h_sb[:, gg * 512 : (gg + 1) * 512]
                )
            mv = ln_pool.tile([P, 2], F32, tag="mv")
            nc.vector.bn_aggr(out=mv, in_=stats)
            sd = ln_pool.tile([P, 1], F32, tag="sd")
            nc.scalar.activation(
                sd, mv[:, 1:2], mybir.ActivationFunctionType.Sqrt, bias=eps_t
            )
            rs = ln_pool.tile([P, 1], F32, tag="rs")
            nc.vector.reciprocal(rs, sd)
            t1 = ln_pool.tile([P, H], F32, tag="t1")
            nc.vector.scalar_tensor_tensor(
                out=t1, in0=h_sb, scalar=mv[:, 0:1], in1=gamma_bc, op0=SUB, op1=MULT
            )
            t2_ = ln_pool.tile([P, H], F32, tag="t2_")
            nc.gpsimd.tensor_scalar(
                out=t2_, in0=t1, scalar1=rs, scalar2=None, op0=MULT
            )
            ob = ln_pool.tile([P, H], F32, tag="ob")
            nc.gpsimd.tensor_tensor(out=ob, in0=t2_, in1=beta_bc, op=ADD)
            nc.sync.dma_start(out=out_d[st * P : (st + 1) * P, :], in_=ob)

        for _pool in (ps_ctx, ps_s, ps_mm, ln_pool, r_pool, pt_pool, kv_pool,
                      w_pool, xq_pool, ctxT_pool, wv_pool, va_pool, xT_pool,
                      consts):
            _pool.release()

    _split_excess_waits(nc)
    return nc


_NC = None


def _get_nc():
    global _NC
    if _NC is None:
        _NC = build_nc()
    return _NC


def _in_maps(hidden_states, attention_mask, Wq, bq, Wk, bk, Wv, bv, Wo, bo, gamma, beta):
    f8 = ml_dtypes.float8_e4m3
    hs = np.asarray(hidden_states, dtype=np.float32)
    am = np.asarray(attention_mask, dtype=np.float32).reshape(2, S)
    Wo_f = np.asarray(Wo, dtype=np.float32)

    def pair_w(w):
        w = np.asarray(w, dtype=np.float32) * WS
        return np.ascontiguousarray(
            w.reshape(HC, P, NPAIR, P).transpose(2, 1, 0, 3).reshape(NPAIR, P, H)
        ).astype(f8)

    wq_t, wk_t = pair_w(Wq), pair_w(Wk)
    wv_t = np.ascontiguousarray(
        (np.asarray(Wv, dtype=np.float32) * WS)
        .reshape(HC, P, 2, 512)
        .transpose(2, 1, 0, 3)
        .reshape(2, P, HC * 512)
    ).astype(f8)
    wo_t = np.ascontiguousarray(
        (Wo_f * WS).reshape(HC, P, H).transpose(1, 0, 2).reshape(P, HC * H)
    ).astype(f8)
    bqk = np.ascontiguousarray(
        np.concatenate(
            [
                (np.asarray(b, dtype=np.float32) * WS).reshape(NPAIR, P).T
                for b in (bq, bk)
            ],
            axis=1,
        )
    )
    g_c = np.ascontiguousarray(np.asarray(gamma, dtype=np.float32))
    be_c = np.ascontiguousarray(np.asarray(beta, dtype=np.float32))
    # residual folds: x + bo + bv @ Wo
    res_c = (
        np.asarray(bo, dtype=np.float32)
        + np.asarray(bv, dtype=np.float32) @ Wo_f
    )

    maps = []
    for core in range(8):
        b, j = core // 4, core % 4
        # roll the sequence so this core's query block is always cols [0, 512);
        # attention sums over all keys, so key order is irrelevant as long as
        # the multiplicative mask em is rolled identically.
        xs = np.roll(hs[b], -j * SQ, axis=0)
        ms = np.roll(am[b], -j * SQ)
        xT = np.ascontiguousarray(
            xs.T.reshape(HC, P, S).transpose(1, 0, 2)
        ).astype(f8)
        maps.append(
            {
                "xT": xT,
                "xq": np.ascontiguousarray(xs[0:SQ, :] + res_c),
                "wq": wq_t,
                "wk": wk_t,
                "wv": wv_t,
                "wo": wo_t,
                "bqk": bqk,
                "gamma": g_c,
                "beta": be_c,
                "em": np.ascontiguousarray(np.exp(ms).reshape(NKT, P).T),
            }
        )
    return maps


def run(trace=False, **inputs):
    nc = _get_nc()
    maps = _in_maps(**inputs)
    res = run_bass_kernel_spmd(nc, maps, core_ids=list(range(8)), trace=trace)
    out = np.empty((2, S, H), dtype=np.float32)
    for core in range(8):
        b, j = core // 4, core % 4
        out[b, j * SQ : (j + 1) * SQ, :] = res.results[core]["out"]
    return out, res


def kernel(**inputs):
    out, _ = run(trace=False, **inputs)
    return out


# revision 3
# speedup vs baseline: 1.0226x; 1.0226x over previous
"""BertAttention Trainium2 kernel v2 (8 NeuronCores, SPMD, no collectives).

Sharding: DP over batch (2) x sequence-parallel over 512-row query blocks (4).
All heavy matmuls run fp8e4 DoubleRow (2 contraction subtiles per instruction,
0.5 cyc/output-column):

  - host sends xT pre-transposed fp8 (sequence rolled so this core's query
    block is cols [0,512)), weights pre-tiled fp8 scaled by 32 (keeps
    W ~N(0,0.02^2) out of the fp8 subnormal range), biases f32.
  - attention_mask folded on host: em = exp(mask) scales the vaug rows
    (data cols and the denominator ones-col), so exp needs no bias;
    bv and bo fold into the residual (xq += bo + bv @ Wo).
  - Q/K proj per head-pair in [d, seq] orientation; evac adds 32*bias, casts
    fp8 (DVE). scores contract d=64 as two 32-partition DoubleRow subtiles
    via DMA-rearranged kdr/qdr. V computed in natural [sk, d] orientation,
    4 head-pairs per PSUM; evac scales by em[sk], casts fp8 into vaug
    [128, j, 96] tiles (col 64 = em, cols 65..95 zero pad - DoubleRow wants
    M % 32 == 0). ctx PSUM row 64 accumulates the softmax denominator.
  - softmax exp on Act engine; a subset of tiles is offloaded to DVE via a
    Schraudolph-style trick: u8 = round(8*log2(pt)+56) bitcast to fp8e4.
  - ctx /= den via fast reciprocal + ones(1.0) K=1 matmul broadcast; the
    fp8 cast multiplies by 32 (scalar_tensor_tensor) so ctxT = 1024*ctx.
  - out-proj evac: Act scale-copy 2^-15 + DVE residual add; LayerNorm via
    bn_stats/aggr, Act Rsqrt, fused DVE stt + Pool tensor_scalar/add tail.
"""

import numpy as np
import ml_dtypes

import bass_rust as _br
import concourse.bass as bass
import concourse.tile as tile
from concourse import mybir
from concourse.bass_utils import run_bass_kernel_spmd

F32 = mybir.dt.float32
F32R = mybir.dt.float32r
F8 = mybir.dt.float8e4
U8 = mybir.dt.uint8
DR = mybir.MatmulPerfMode.DoubleRow
ADD = mybir.AluOpType.add
MULT = mybir.AluOpType.mult
SUB = mybir.AluOpType.subtract

S = 2048
H = 1024
P = 128
SQ = 512          # query rows per core
NKT = S // P      # 16 sk tiles
HC = H // P       # 8 h-chunks
NPAIR = 8         # head pairs
WS = 32.0         # fp8 weight scale
EPS = 1e-12
EXP_SCALE = 0.125 / (WS * WS)     # 2^-13
OUT_SCALE = 1.0 / (WS * WS * WS)  # 2^-15
# Schraudolph-on-DVE offload: these sk-tiles' exps run on DVE as
# u8 = s * SCH_A + SCH_B bitcast fp8e4 (exp2 piecewise-linear approx).
# Per-pair sets sized to balance DVE load (pairs 0-3 also carry V evacs).
SCH_BY_PAIR = (
    (),
    (7, 10, 13),
    (7, 10, 13),
    (7, 10, 13),
    (3, 7, 11, 14),
    (3, 7, 11, 14),
    (3, 7, 11, 14),
    (2, 5, 8, 11, 14),
)
SCH_A = 8 * 1.4426950408889634 * EXP_SCALE
SCH_B = 56.0
# V-projection emission schedule: group 0 just-in-time inside pair 0;
# group 1 spread over pairs 1-3 (list of t per (pair, t2 slot)).
V_SCHED = {
    1: {0: (0, 1), 1: (2, 3), 2: (4, 5)},
    2: {0: (6, 7), 1: (8, 9), 2: (10,)},
    3: {0: (11, 12), 1: (13, 14), 2: (15,)},
}

_wait_ctr = [0]


def _split_excess_waits(nc, limit=1):
    """walrus in this container rejects >1-2 sem waits on several opcode
    structs; move excess waits onto same-engine NoOps inserted just before."""
    for f in nc.m.functions:
        for bb in f.blocks:
            insts = bb.instructions
            out = []
            dirty = False
            for inst in insts:
                si = inst.sync_info
                waits = list(si.on_wait) if si and si.on_wait else []
                if len(waits) > limit and inst.engine != mybir.EngineType.Unassigned:
                    for i in range(0, len(waits) - limit, limit):
                        _wait_ctr[0] += 1
                        nop = _br.InstNoOp(
                            name=f"I-waitsplit-{_wait_ctr[0]}", ins=[], outs=[]
                        )
                        nop.engine = inst.engine
                        nop.sync_info = mybir.SyncInfo(
                            on_wait=waits[i : i + limit], on_update=[]
                        )
                        out.append(nop)
                    si.on_wait = waits[len(waits) - limit :]
                    dirty = True
                out.append(inst)
            if dirty:
                bb.instructions = out
    return nc


def _ap(t, off, dims):
    """Strided AP over a tile's partition range: dims = [[stride, count], ...]."""
    return bass.AP(
        tensor=t.tensor,
        offset=t.offset + off,
        ap=[list(t.ap[0])] + [list(d) for d in dims],
    )


# vaug layout per partition: [t2(8), j(2), m(8), h(2), c(96)]
VA_C = 96
VA_H = VA_C            # stride of h
VA_M = 2 * VA_C        # 192
VA_J = 8 * VA_M        # 1536
VA_T2 = 2 * VA_J       # 3072
VA_TOTAL = 8 * VA_T2   # 24576


def build_nc():
    nc = bass.Bass()

    xT_d = nc.dram_tensor("xT", [P, HC, S], F8, kind="ExternalInput")
    xq_d = nc.dram_tensor("xq", [SQ, H], F32, kind="ExternalInput")  # +bo+bv@Wo
    wq_d = nc.dram_tensor("wq", [NPAIR, P, HC * P], F8, kind="ExternalInput")
    wk_d = nc.dram_tensor("wk", [NPAIR, P, HC * P], F8, kind="ExternalInput")
    wv_d = nc.dram_tensor("wv", [2, P, HC * 512], F8, kind="ExternalInput")
    wo_d = nc.dram_tensor("wo", [P, HC * H], F8, kind="ExternalInput")
    bqk_d = nc.dram_tensor("bqk", [P, 16], F32, kind="ExternalInput")  # 32*(bq|bk)
    gamma_d = nc.dram_tensor("gamma", [H], F32, kind="ExternalInput")
    beta_d = nc.dram_tensor("beta", [H], F32, kind="ExternalInput")
    em_d = nc.dram_tensor("em", [P, NKT], F32, kind="ExternalInput")  # exp(mask)
    out_d = nc.dram_tensor("out", [SQ, H], F32, kind="ExternalOutput")

    with tile.TileContext(nc) as tc, nc.allow_low_precision(
        reason="fp8 DoubleRow matmuls; accumulation stays fp32 in PSUM"
    ):
        consts = tc.alloc_tile_pool(name="consts", bufs=1)
        xT_pool = tc.alloc_tile_pool(name="xT", bufs=1)
        va_pool = tc.alloc_tile_pool(name="va", bufs=1)
        wv_pool = tc.alloc_tile_pool(name="wv", bufs=1)
        ctxT_pool = tc.alloc_tile_pool(name="ctxT", bufs=1)
        xq_pool = tc.alloc_tile_pool(name="xq", bufs=1)
        w_pool = tc.alloc_tile_pool(name="w", bufs=3)
        kv_pool = tc.alloc_tile_pool(name="kv", bufs=4)
        pt_pool = tc.alloc_tile_pool(name="pt", bufs=6)
        r_pool = tc.alloc_tile_pool(name="r", bufs=4)
        ln_pool = tc.alloc_tile_pool(name="ln", bufs=2)
        ps_mm = tc.alloc_tile_pool(name="ps_mm", bufs=2, space="PSUM")
        ps_s = tc.alloc_tile_pool(name="ps_s", bufs=2, space="PSUM")
        ps_ctx = tc.alloc_tile_pool(name="ps_ctx", bufs=2, space="PSUM")

        # ---- critical-path DMAs first (HWDGE is ~620ns PER DMA op and the
        # transfers mostly serialize, so few, large, well-ordered DMAs
        # matter): xT query cols, then wq0/wk0 ----
        xT_all = xT_pool.tile([P, HC * S], F8, name="xT_all", tag="xT_all")
        xT_v = xT_all.rearrange("p (c s) -> p c s", c=HC)
        # query-block columns (0..512) of every chunk in one strided DMA:
        # unblocks the q projection and the first k block early
        nc.sync.dma_start(out=xT_v[:, :, 0:512], in_=xT_d[:, :, 0:512])
        wq0 = w_pool.tile([P, HC * P], F8, tag="wq_m")
        nc.sync.dma_start(out=wq0, in_=wq_d[0])
        wk0 = w_pool.tile([P, HC * P], F8, tag="wk_m")
        nc.sync.dma_start(out=wk0, in_=wk_d[0])
        bqk = consts.tile([P, 16], F32, tag="bqk")
        nc.sync.dma_start(out=bqk, in_=bqk_d[:, :])
        em = consts.tile([P, NKT], F32, tag="em")
        nc.sync.dma_start(out=em, in_=em_d[:, :])

        # ---- small consts / VA init (Pool) ----
        eps_t = consts.tile([P, 1], F32, tag="eps")
        nc.vector.memset(eps_t, EPS)
        ones32 = consts.tile([1, 64], F32, tag="ones32")
        nc.vector.memset(ones32, 1.0)
        ones32_r = ones32.bitcast(F32R)
        zeros16 = consts.tile([P, 16], F32, tag="zeros16")
        nc.gpsimd.memset(zeros16, 0.0)

        VA = va_pool.tile([P, VA_TOTAL], F8, name="VA", tag="VA")
        for t2 in range(8):
            for j in range(2):
                base = VA_T2 * t2 + VA_J * j
                # zero the pad block (cols 64..95)
                nc.gpsimd.memset(
                    _ap(VA, base + 64, [[VA_M, 8], [VA_H, 2], [1, 32]]), 0.0
                )
        for t in range(NKT):
            # denominator col (64) = em[:, t] per (m, h)
            base = VA_T2 * (t // 2) + VA_J * (t % 2) + 64
            nc.gpsimd.tensor_scalar(
                out=_ap(VA, base, [[VA_M, 8], [VA_H, 2]]),
                in0=_ap(zeros16, 0, [[2, 8], [1, 2]]),
                scalar1=em[:, t : t + 1],
                scalar2=None,
                op0=ADD,
            )

        ctxT_all = ctxT_pool.tile([P, NPAIR * SQ], F8, name="ctxT", tag="ctxT")

        # ---- AP helpers ----
        def xt_rhs(c2, s0, ns):
            return _ap(xT_all, 2 * c2 * S + s0, [[S, 2], [1, ns]])

        def xt_lhsT(c2, t):
            return _ap(xT_all, 2 * c2 * S + t * P, [[S, 2], [1, P]])

        def w_lhsT(w, c2):
            return _ap(w, 2 * c2 * P, [[P, 2], [1, P]])

        def wv_rhs(g, c2):
            return _ap(wv_g[g], 2 * c2 * 512, [[512, 2], [1, 512]])

        def va_lhsT(t2, m, h):
            return _ap(VA, VA_T2 * t2 + VA_M * m + VA_H * h, [[VA_J, 2], [1, VA_C]])

        def va_dst(t, g):
            return _ap(
                VA,
                VA_T2 * (t // 2) + VA_J * (t % 2) + VA_M * (4 * g),
                [[VA_M, 4], [VA_H, 2], [1, 64]],
            )

        def kdr_lhsT(kdr, h, t):
            return _ap(kdr, 4096 * h + t * P, [[2048, 2], [1, P]])

        def qdr_rhs(qdr, h):
            return _ap(qdr, 1024 * h, [[512, 2], [1, 512]])

        def pt_rhs(pt, h):
            return _ap(pt, 512 * h, [[1024, 2], [1, 512]])

        def ctxT_lhsT(c2, st):
            return _ap(ctxT_all, 2 * c2 * SQ + st * P, [[SQ, 2], [1, P]])

        # ---- per-pair Q/K projection + DoubleRow rearrangements ----
        def emit_w_dma(m):
            wq_m = w_pool.tile([P, HC * P], F8, tag="wq_m")
            nc.sync.dma_start(out=wq_m, in_=wq_d[m])
            wk_m = w_pool.tile([P, HC * P], F8, tag="wk_m")
            nc.sync.dma_start(out=wk_m, in_=wk_d[m])
            return wq_m, wk_m

        def emit_q(m, wq_m):
            ps = ps_mm.tile([P, 512], F32, name="ps", tag="ps")
            for c2 in range(4):
                nc.tensor.matmul(
                    ps,
                    w_lhsT(wq_m, c2),
                    xt_rhs(c2, 0, 512),
                    start=(c2 == 0),
                    stop=(c2 == 3),
                    perf_mode=DR,
                )
            q_sb = kv_pool.tile([P, 512], F8, tag="q_sb")
            nc.vector.tensor_scalar_add(q_sb, ps, bqk[:, m : m + 1])
            qdr = kv_pool.tile([32, 2048], F8, tag="qdr")
            for h in range(2):
                for j in range(2):
                    nc.sync.dma_start(
                        out=qdr[:, 1024 * h + 512 * j : 1024 * h + 512 * (j + 1)],
                        in_=q_sb[64 * h + 32 * j : 64 * h + 32 * j + 32, :],
                    )
            return qdr

        def emit_k_block(m, wk_m, k_sb, n):
            ps = ps_mm.tile([P, 512], F32, name="ps", tag="ps")
            for c2 in range(4):
                nc.tensor.matmul(
                    ps,
                    w_lhsT(wk_m, c2),
                    xt_rhs(c2, n * 512, 512),
                    start=(c2 == 0),
                    stop=(c2 == 3),
                    perf_mode=DR,
                )
            nc.vector.tensor_scalar_add(
                k_sb[:, n * 512 : (n + 1) * 512], ps, bqk[:, 8 + m : 9 + m]
            )

        def emit_kdr_dma(k_sb, kdr, c0, c1):
            for h in range(2):
                for j in range(2):
                    nc.sync.dma_start(
                        out=kdr[:, 4096 * h + 2048 * j + c0 : 4096 * h + 2048 * j + c1],
                        in_=k_sb[64 * h + 32 * j : 64 * h + 32 * j + 32, c0:c1],
                    )

        def emit_qk_proj(m, wq_m=None, wk_m=None):
            if wq_m is None:
                wq_m, wk_m = emit_w_dma(m)
            qdr = emit_q(m, wq_m)
            k_sb = kv_pool.tile([P, S], F8, tag="k_sb")
            kdr = kv_pool.tile([32, 8192], F8, tag="kdr")
            emit_k_block(m, wk_m, k_sb, 0)
            emit_kdr_dma(k_sb, kdr, 0, 512)
            # startup only (this path only runs for pair 0): wv group 0 must
            # beat the first ctx tiles, then the rest of xT for k blocks 1-3
            wvt0 = wv_pool.tile([P, HC * 512], F8, name="wv0", tag="wv0")
            nc.sync.dma_start(out=wvt0, in_=wv_d[0])
            wv_g.append(wvt0)
            nc.sync.dma_start(out=xT_v[:, :, 512:1280], in_=xT_d[:, :, 512:1280])
            nc.sync.dma_start(out=xT_v[:, :, 1280:S], in_=xT_d[:, :, 1280:S])
            for n in range(1, 4):
                emit_k_block(m, wk_m, k_sb, n)
            emit_kdr_dma(k_sb, kdr, 512, 2048)
            return qdr, kdr

        def emit_v(g, t):
            ps = ps_mm.tile([P, 512], F32, name="vps", tag="ps")
            for c2 in range(4):
                nc.tensor.matmul(
                    ps,
                    xt_lhsT(c2, t),
                    wv_rhs(g, c2),
                    start=(c2 == 0),
                    stop=(c2 == 3),
                    perf_mode=DR,
                )
            nc.vector.tensor_scalar(
                out=va_dst(t, g),
                in0=_ap(ps, 0, [[P, 4], [64, 2], [1, 64]]),
                scalar1=em[:, t : t + 1],
                scalar2=None,
                op0=MULT,
            )

        # ---- main loop ----
        def emit_norm(m, ctx_ps):
            # normalize + fp8 ctxT (= 1024 * ctx); recips first so the DVE
            # chain pipelines across both heads
            rrs = []
            for h in range(2):
                rr = r_pool.tile([1, 512], F32R, tag="rr")
                nc.vector.reciprocal(rr, ctx_ps[h][64:65, :])
                rrs.append(rr)
            bcs = []
            for h in range(2):
                bc_ps = ps_mm.tile([64, 512], F32, name="bc_ps", tag="ps")
                nc.tensor.matmul(bc_ps, ones32_r, rrs[h], start=True, stop=True)
                bcs.append(bc_ps)
            rbs = []
            for h in range(2):
                rb = r_pool.tile([64, 512], F32, tag="rb")
                nc.vector.tensor_copy(rb, bcs[h])
                rbs.append(rb)
            for h in range(2):
                nc.vector.scalar_tensor_tensor(
                    out=ctxT_all[64 * h : 64 * h + 64, m * SQ : (m + 1) * SQ],
                    in0=ctx_ps[h][0:64, :],
                    scalar=WS,
                    in1=rbs[h],
                    op0=MULT,
                    op1=MULT,
                )

        wv_g = []
        dr_cur = emit_qk_proj(0, wq0, wk0)
        wvt1 = wv_pool.tile([P, HC * 512], F8, name="wv1", tag="wv1")
        nc.sync.dma_start(out=wvt1, in_=wv_d[1])
        wv_g.append(wvt1)
        prev_norm = None  # (m, ctx_ps) awaiting normalization
        w_next = None
        k_next = None
        for m in range(NPAIR):
            qdr, kdr = dr_cur
            ctx_ps = [
                ps_ctx.tile([P, 512], F32, name=f"ctx{h}", tag="ctx_ps")
                for h in range(2)
            ]

            def emit_ctx(e_t2, e_pt):
                for h in range(2):
                    nc.tensor.matmul(
                        ctx_ps[h][0:96, :],
                        va_lhsT(e_t2, m, h),
                        pt_rhs(e_pt, h),
                        start=(e_t2 == 0),
                        stop=(e_t2 == 7),
                        perf_mode=DR,
                    )

            pend_ctx = []
            for t2 in range(8):
                pt = pt_pool.tile([P, 2048], F8, name="pt", tag="pt")
                for j in range(2):
                    t = 2 * t2 + j
                    s2 = ps_s.tile([P, 1024], F32, name="s2", tag="s2")
                    for h in range(2):
                        nc.tensor.matmul(
                            s2[:, 512 * h : 512 * (h + 1)],
                            kdr_lhsT(kdr, h, t),
                            qdr_rhs(qdr, h),
                            start=True,
                            stop=True,
                            perf_mode=DR,
                        )
                    if t in SCH_BY_PAIR[m]:
                        nc.vector.tensor_scalar(
                            out=pt[:, 1024 * j : 1024 * (j + 1)].bitcast(U8),
                            in0=s2,
                            scalar1=SCH_A,
                            scalar2=SCH_B,
                            op0=MULT,
                            op1=ADD,
                        )
                    else:
                        nc.scalar.activation(
                            pt[:, 1024 * j : 1024 * (j + 1)],
                            s2,
                            mybir.ActivationFunctionType.Exp,
                            scale=EXP_SCALE,
                        )
                # interleaved work while Act chews the exps; t2==2 keeps the
                # DVE norm chain away from the Schraudolph slots
                if t2 == 4 and prev_norm is not None:
                    emit_norm(*prev_norm)
                    prev_norm = None
                if m == 0:
                    emit_v(0, 2 * t2)
                    emit_v(0, 2 * t2 + 1)
                elif m in V_SCHED and t2 in V_SCHED[m]:
                    for tv in V_SCHED[m][t2]:
                        emit_v(1, tv)
                if m < NPAIR - 1:
                    # next pair's proj, spread across t2 slots
                    if t2 == 1:
                        w_next = emit_w_dma(m + 1)
                    elif t2 == 2:
                        q_next = emit_q(m + 1, w_next[0])
                        k_next = (
                            kv_pool.tile([P, S], F8, tag="k_sb", name="k_sb"),
                            kv_pool.tile([32, 8192], F8, tag="kdr", name="kdr"),
                        )
                        dr_next = (q_next, k_next[1])
                    elif t2 in (3, 4, 5, 6):
                        emit_k_block(m + 1, w_next[1], k_next[0], t2 - 3)
                        if t2 == 6:
                            emit_kdr_dma(k_next[0], k_next[1], 0, 2048)
                if t2 == 5 and 2 <= m <= 6:
                    # deferred non-critical input DMAs, spread one pair apart
                    # so they never clump ahead of the next pair's weights
                    if m == 2:
                        xq = []
                    if m <= 5:
                        xqt = xq_pool.tile(
                            [P, H], F32, name=f"xq{m-2}", tag=f"xq{m-2}"
                        )
                        nc.sync.dma_start(
                            out=xqt, in_=xq_d[(m - 2) * P : (m - 1) * P, :]
                        )
                        xq.append(xqt)
                    if m == 3:
                        gamma_bc = consts.tile([P, H], F32, tag="gamma_bc")
                        nc.sync.dma_start(
                            out=gamma_bc, in_=gamma_d[:].partition_broadcast(P)
                        )
                    elif m == 4:
                        beta_bc = consts.tile([P, H], F32, tag="beta_bc")
                        nc.sync.dma_start(
                            out=beta_bc, in_=beta_d[:].partition_broadcast(P)
                        )
                    elif m == 6:
                        wo_sb = wv_pool.tile([P, HC * H], F8, name="wo", tag="wo")
                        nc.sync.dma_start(out=wo_sb, in_=wo_d[:, :])
                # defer ctx emission by 2 slots so a pending WAR on the
                # ctx accumulators (prev pair's norm) can't head-of-line
                # block the scores stream on the in-order PE
                pend_ctx.append((t2, pt))
                if len(pend_ctx) > 2:
                    emit_ctx(*pend_ctx.pop(0))
            for e in pend_ctx:
                emit_ctx(*e)
            prev_norm = (m, ctx_ps)
            dr_cur = dr_next
        emit_norm(*prev_norm)

        def wo_rhs(c2, nch):
            return _ap(wo_sb, 2 * c2 * H + nch * 512, [[H, 2], [1, 512]])

        # ---- output projection + residual + LayerNorm ----
        for st in range(SQ // P):
            h_sb = ln_pool.tile([P, H], F32, tag="h_sb")
            for nch in range(2):
                ps = ps_mm.tile([P, 512], F32, name="ops", tag="ps")
                for c2 in range(4):
                    nc.tensor.matmul(
                        ps,
                        ctxT_lhsT(c2, st),
                        wo_rhs(c2, nch),
                        start=(c2 == 0),
                        stop=(c2 == 3),
                        perf_mode=DR,
                    )
                h0 = ln_pool.tile([P, 512], F32, tag="h0")
                nc.scalar.mul(h0, ps, OUT_SCALE)
                nc.vector.tensor_add(
                    h_sb[:, nch * 512 : (nch + 1) * 512],
                    h0,
                    xq[st][:, nch * 512 : (nch + 1) * 512],
                )
            stats = ln_pool.tile([P, 2, 6], F32, tag="stats")
            for gg in range(2):
                nc.vector.bn_stats(
                    out=stats[:, gg, :], in_=h_sb[:, gg * 512 : (gg + 1) * 512]
                )
            mv = ln_pool.tile([P, 2], F32, tag="mv")
            nc.vector.bn_aggr(out=mv, in_=stats)
            sd = ln_pool.tile([P, 1], F32, tag="sd")
            nc.scalar.activation(
                sd, mv[:, 1:2], mybir.ActivationFunctionType.Sqrt, bias=eps_t
            )
            rs = ln_pool.tile([P, 1], F32, tag="rs")
            nc.vector.reciprocal(rs, sd)
            t1 = ln_pool.tile([P, H], F32, tag="t1")
            nc.vector.scalar_tensor_tensor(
                out=t1, in0=h_sb, scalar=mv[:, 0:1], in1=gamma_bc, op0=SUB, op1=MULT
            )
            t2_ = ln_pool.tile([P, H], F32, tag="t2_")
            nc.gpsimd.tensor_scalar(
                out=t2_, in0=t1, scalar1=rs, scalar2=None, op0=MULT
            )
            ob = ln_pool.tile([P, H], F32, tag="ob")
            nc.gpsimd.tensor_tensor(out=ob, in0=t2_, in1=beta_bc, op=ADD)
            nc.sync.dma_start(out=out_d[st * P : (st + 1) * P, :], in_=ob)

        for _pool in (ps_ctx, ps_s, ps_mm, ln_pool, r_pool, pt_pool, kv_pool,
                      w_pool, xq_pool, ctxT_pool, wv_pool, va_pool, xT_pool,
                      consts):
            _pool.release()

    _split_excess_waits(nc)
    return nc


_NC = None


def _get_nc():
    global _NC
    if _NC is None:
        _NC = build_nc()
    return _NC


def _in_maps(hidden_states, attention_mask, Wq, bq, Wk, bk, Wv, bv, Wo, bo, gamma, beta):
    f8 = ml_dtypes.float8_e4m3
    hs = np.asarray(hidden_states, dtype=np.float32)
    am = np.asarray(attention_mask, dtype=np.float32).reshape(2, S)
    Wo_f = np.asarray(Wo, dtype=np.float32)

    def pair_w(w):
        w = np.asarray(w, dtype=np.float32) * WS
        return np.ascontiguousarray(
            w.reshape(HC, P, NPAIR, P).transpose(2, 1, 0, 3).reshape(NPAIR, P, H)
        ).astype(f8)

    wq_t, wk_t = pair_w(Wq), pair_w(Wk)
    wv_t = np.ascontiguousarray(
        (np.asarray(Wv, dtype=np.float32) * WS)
        .reshape(HC, P, 2, 512)
        .transpose(2, 1, 0, 3)
        .reshape(2, P, HC * 512)
    ).astype(f8)
    wo_t = np.ascontiguousarray(
        (Wo_f * WS).reshape(HC, P, H).transpose(1, 0, 2).reshape(P, HC * H)
    ).astype(f8)
    bqk = np.ascontiguousarray(
        np.concatenate(
            [
                (np.asarray(b, dtype=np.float32) * WS).reshape(NPAIR, P).T
                for b in (bq, bk)
            ],
            axis=1,
        )
    )
    g_c = np.ascontiguousarray(np.asarray(gamma, dtype=np.float32))
    be_c = np.ascontiguousarray(np.asarray(beta, dtype=np.float32))
    # residual folds: x + bo + bv @ Wo
    res_c = (
        np.asarray(bo, dtype=np.float32)
        + np.asarray(bv, dtype=np.float32) @ Wo_f
    )

    maps = []
    for core in range(8):
        b, j = core // 4, core % 4
        # roll the sequence so this core's query block is always cols [0, 512);
        # attention sums over all keys, so key order is irrelevant as long as
        # the multiplicative mask em is rolled identically.
        xs = np.roll(hs[b], -j * SQ, axis=0)
        ms = np.roll(am[b], -j * SQ)
        xT = np.ascontiguousarray(
            xs.T.reshape(HC, P, S).transpose(1, 0, 2)
        ).astype(f8)
        maps.append(
            {
                "xT": xT,
                "xq": np.ascontiguousarray(xs[0:SQ, :] + res_c),
                "wq": wq_t,
                "wk": wk_t,
                "wv": wv_t,
                "wo": wo_t,
                "bqk": bqk,
                "gamma": g_c,
                "beta": be_c,
                "em": np.ascontiguousarray(np.exp(ms).reshape(NKT, P).T),
            }
        )
    return maps


def run(trace=False, **inputs):
    nc = _get_nc()
    maps = _in_maps(**inputs)
    res = run_bass_kernel_spmd(nc, maps, core_ids=list(range(8)), trace=trace)
    out = np.empty((2, S, H), dtype=np.float32)
    for core in range(8):
        b, j = core // 4, core % 4
        out[b, j * SQ : (j + 1) * SQ, :] = res.results[core]["out"]
    return out, res


def kernel(**inputs):
    out, _ = run(trace=False, **inputs)
    return out


# revision 4
# speedup vs baseline: 1.0343x; 1.0115x over previous
"""BertAttention Trainium2 kernel v2 (8 NeuronCores, SPMD, no collectives).

Sharding: DP over batch (2) x sequence-parallel over 512-row query blocks (4).
All heavy matmuls run fp8e4 DoubleRow (2 contraction subtiles per instruction,
0.5 cyc/output-column):

  - host sends xT pre-transposed fp8 (sequence rolled so this core's query
    block is cols [0,512)), weights pre-tiled fp8 scaled by 32 (keeps
    W ~N(0,0.02^2) out of the fp8 subnormal range), biases f32.
  - attention_mask folded on host: em = exp(mask) scales the vaug rows
    (data cols and the denominator ones-col), so exp needs no bias;
    bv and bo fold into the residual (xq += bo + bv @ Wo).
  - Q/K proj per head-pair in [d, seq] orientation; evac adds 32*bias, casts
    fp8 (DVE). scores contract d=64 as two 32-partition DoubleRow subtiles
    via DMA-rearranged kdr/qdr. V computed in natural [sk, d] orientation,
    4 head-pairs per PSUM; evac scales by em[sk], casts fp8 into vaug
    [128, j, 96] tiles (col 64 = em, cols 65..95 zero pad - DoubleRow wants
    M % 32 == 0). ctx PSUM row 64 accumulates the softmax denominator.
  - softmax exp on Act engine; a subset of tiles is offloaded to DVE via a
    Schraudolph-style trick: u8 = round(8*log2(pt)+56) bitcast to fp8e4.
  - ctx /= den via fast reciprocal + ones(1.0) K=1 matmul broadcast; the
    fp8 cast multiplies by 32 (scalar_tensor_tensor) so ctxT = 1024*ctx.
  - out-proj evac: Act scale-copy 2^-15 + DVE residual add; LayerNorm via
    bn_stats/aggr, Act Rsqrt, fused DVE stt + Pool tensor_scalar/add tail.
"""

import numpy as np
import ml_dtypes

import bass_rust as _br
import concourse.bass as bass
import concourse.tile as tile
from concourse import mybir
from concourse.bass_utils import run_bass_kernel_spmd

F32 = mybir.dt.float32
F32R = mybir.dt.float32r
F8 = mybir.dt.float8e4
U8 = mybir.dt.uint8
DR = mybir.MatmulPerfMode.DoubleRow
ADD = mybir.AluOpType.add
MULT = mybir.AluOpType.mult
SUB = mybir.AluOpType.subtract

S = 2048
H = 1024
P = 128
SQ = 512          # query rows per core
NKT = S // P      # 16 sk tiles
HC = H // P       # 8 h-chunks
NPAIR = 8         # head pairs
WS = 32.0         # fp8 weight scale
EPS = 1e-12
EXP_SCALE = 0.125 / (WS * WS)     # 2^-13
OUT_SCALE = 1.0 / (WS * WS * WS)  # 2^-15
# Schraudolph-on-DVE offload: these sk-tiles' exps run on DVE as
# u8 = s * SCH_A + SCH_B bitcast fp8e4 (exp2 piecewise-linear approx).
# Per-pair sets sized to balance DVE load (pairs 0-3 also carry V evacs).
SCH_BY_PAIR = (
    (),
    (7, 10, 13),
    (7, 10, 13),
    (7, 10, 13),
    (3, 7, 11, 14),
    (3, 7, 11, 14),
    (3, 7, 11, 14),
    (2, 5, 8, 11, 14),
)
SCH_A = 8 * 1.4426950408889634 * EXP_SCALE
SCH_B = 56.0
# V-projection emission schedule: group 0 just-in-time inside pair 0;
# group 1 spread over pairs 1-3 (list of t per (pair, t2 slot)).
V_SCHED = {
    1: {0: (0, 1), 1: (2, 3), 2: (4, 5)},
    2: {0: (6, 7), 1: (8, 9), 2: (10,)},
    3: {0: (11, 12), 1: (13, 14), 2: (15,)},
}

_wait_ctr = [0]


def _split_excess_waits(nc, limit=1):
    """walrus in this container rejects >1-2 sem waits on several opcode
    structs; move excess waits onto same-engine NoOps inserted just before."""
    for f in nc.m.functions:
        for bb in f.blocks:
            insts = bb.instructions
            out = []
            dirty = False
            for inst in insts:
                si = inst.sync_info
                waits = list(si.on_wait) if si and si.on_wait else []
                if len(waits) > limit and inst.engine != mybir.EngineType.Unassigned:
                    for i in range(0, len(waits) - limit, limit):
                        _wait_ctr[0] += 1
                        nop = _br.InstNoOp(
                            name=f"I-waitsplit-{_wait_ctr[0]}", ins=[], outs=[]
                        )
                        nop.engine = inst.engine
                        nop.sync_info = mybir.SyncInfo(
                            on_wait=waits[i : i + limit], on_update=[]
                        )
                        out.append(nop)
                    si.on_wait = waits[len(waits) - limit :]
                    dirty = True
                out.append(inst)
            if dirty:
                bb.instructions = out
    return nc


def _ap(t, off, dims):
    """Strided AP over a tile's partition range: dims = [[stride, count], ...]."""
    return bass.AP(
        tensor=t.tensor,
        offset=t.offset + off,
        ap=[list(t.ap[0])] + [list(d) for d in dims],
    )


# vaug layout per partition: [t2(8), j(2), m(8), h(2), c(96)]
VA_C = 96
VA_H = VA_C            # stride of h
VA_M = 2 * VA_C        # 192
VA_J = 8 * VA_M        # 1536
VA_T2 = 2 * VA_J       # 3072
VA_TOTAL = 8 * VA_T2   # 24576


def build_nc():
    nc = bass.Bass()

    xT_d = nc.dram_tensor("xT", [P, HC, S], F8, kind="ExternalInput")
    xq_d = nc.dram_tensor("xq", [SQ, H], F32, kind="ExternalInput")  # +bo+bv@Wo
    wq_d = nc.dram_tensor("wq", [NPAIR, P, HC * P], F8, kind="ExternalInput")
    wk_d = nc.dram_tensor("wk", [NPAIR, P, HC * P], F8, kind="ExternalInput")
    wv_d = nc.dram_tensor("wv", [2, P, HC * 512], F8, kind="ExternalInput")
    wo_d = nc.dram_tensor("wo", [P, HC * H], F8, kind="ExternalInput")
    bqk_d = nc.dram_tensor("bqk", [P, 16], F32, kind="ExternalInput")  # 32*(bq|bk)
    gamma_d = nc.dram_tensor("gamma", [H], F32, kind="ExternalInput")
    beta_d = nc.dram_tensor("beta", [H], F32, kind="ExternalInput")
    em_d = nc.dram_tensor("em", [P, NKT], F32, kind="ExternalInput")  # exp(mask)
    out_d = nc.dram_tensor("out", [SQ, H], F32, kind="ExternalOutput")

    with tile.TileContext(nc) as tc, nc.allow_low_precision(
        reason="fp8 DoubleRow matmuls; accumulation stays fp32 in PSUM"
    ):
        consts = tc.alloc_tile_pool(name="consts", bufs=1)
        xT_pool = tc.alloc_tile_pool(name="xT", bufs=1)
        va_pool = tc.alloc_tile_pool(name="va", bufs=1)
        wv_pool = tc.alloc_tile_pool(name="wv", bufs=1)
        ctxT_pool = tc.alloc_tile_pool(name="ctxT", bufs=1)
        xq_pool = tc.alloc_tile_pool(name="xq", bufs=1)
        w_pool = tc.alloc_tile_pool(name="w", bufs=3)
        kv_pool = tc.alloc_tile_pool(name="kv", bufs=4)
        pt_pool = tc.alloc_tile_pool(name="pt", bufs=6)
        r_pool = tc.alloc_tile_pool(name="r", bufs=4)
        ln_pool = tc.alloc_tile_pool(name="ln", bufs=2)
        ps_mm = tc.alloc_tile_pool(name="ps_mm", bufs=2, space="PSUM")
        ps_s = tc.alloc_tile_pool(name="ps_s", bufs=2, space="PSUM")
        ps_ctx = tc.alloc_tile_pool(name="ps_ctx", bufs=2, space="PSUM")

        # ---- critical-path DMAs first (HWDGE is ~620ns PER DMA op and the
        # transfers mostly serialize, so few, large, well-ordered DMAs
        # matter): xT query cols, then wq0/wk0 ----
        xT_all = xT_pool.tile([P, HC * S], F8, name="xT_all", tag="xT_all")
        xT_v = xT_all.rearrange("p (c s) -> p c s", c=HC)
        # query-block columns (0..512) of every chunk in one strided DMA:
        # unblocks the q projection and the first k block early
        nc.sync.dma_start(out=xT_v[:, :, 0:512], in_=xT_d[:, :, 0:512])
        wq0 = w_pool.tile([P, HC * P], F8, tag="wq_m")
        nc.sync.dma_start(out=wq0, in_=wq_d[0])
        wk0 = w_pool.tile([P, HC * P], F8, tag="wk_m")
        nc.sync.dma_start(out=wk0, in_=wk_d[0])
        bqk = consts.tile([P, 16], F32, tag="bqk")
        nc.sync.dma_start(out=bqk, in_=bqk_d[:, :])
        em = consts.tile([P, NKT], F32, tag="em")
        nc.sync.dma_start(out=em, in_=em_d[:, :])

        # ---- small consts / VA init (Pool) ----
        eps_t = consts.tile([P, 1], F32, tag="eps")
        nc.vector.memset(eps_t, EPS)
        ones32 = consts.tile([1, 64], F32, tag="ones32")
        nc.vector.memset(ones32, 1.0)
        ones32_r = ones32.bitcast(F32R)
        zeros16 = consts.tile([P, 16], F32, tag="zeros16")
        nc.gpsimd.memset(zeros16, 0.0)

        VA = va_pool.tile([P, VA_TOTAL], F8, name="VA", tag="VA")
        for t2 in range(8):
            for j in range(2):
                base = VA_T2 * t2 + VA_J * j
                # zero the pad block (cols 64..95)
                nc.gpsimd.memset(
                    _ap(VA, base + 64, [[VA_M, 8], [VA_H, 2], [1, 32]]), 0.0
                )
        for t in range(NKT):
            # denominator col (64) = em[:, t] per (m, h)
            base = VA_T2 * (t // 2) + VA_J * (t % 2) + 64
            nc.gpsimd.tensor_scalar(
                out=_ap(VA, base, [[VA_M, 8], [VA_H, 2]]),
                in0=_ap(zeros16, 0, [[2, 8], [1, 2]]),
                scalar1=em[:, t : t + 1],
                scalar2=None,
                op0=ADD,
            )

        ctxT_all = ctxT_pool.tile([P, NPAIR * SQ], F8, name="ctxT", tag="ctxT")

        # ---- AP helpers ----
        def xt_rhs(c2, s0, ns):
            return _ap(xT_all, 2 * c2 * S + s0, [[S, 2], [1, ns]])

        def xt_lhsT(c2, t):
            return _ap(xT_all, 2 * c2 * S + t * P, [[S, 2], [1, P]])

        def w_lhsT(w, c2):
            return _ap(w, 2 * c2 * P, [[P, 2], [1, P]])

        def wv_rhs(g, c2):
            return _ap(wv_g[g], 2 * c2 * 512, [[512, 2], [1, 512]])

        def va_lhsT(t2, m, h):
            return _ap(VA, VA_T2 * t2 + VA_M * m + VA_H * h, [[VA_J, 2], [1, VA_C]])

        def va_dst(t, g):
            return _ap(
                VA,
                VA_T2 * (t // 2) + VA_J * (t % 2) + VA_M * (4 * g),
                [[VA_M, 4], [VA_H, 2], [1, 64]],
            )

        def kdr_lhsT(kdr, h, t):
            return _ap(kdr, 4096 * h + t * P, [[2048, 2], [1, P]])

        def qdr_rhs(qdr, h):
            return _ap(qdr, 1024 * h, [[512, 2], [1, 512]])

        def pt_rhs(pt, h):
            return _ap(pt, 512 * h, [[1024, 2], [1, 512]])

        def ctxT_lhsT(c2, st):
            return _ap(ctxT_all, 2 * c2 * SQ + st * P, [[SQ, 2], [1, P]])

        # ---- per-pair Q/K projection + DoubleRow rearrangements ----
        def emit_w_dma(m):
            wq_m = w_pool.tile([P, HC * P], F8, tag="wq_m")
            nc.sync.dma_start(out=wq_m, in_=wq_d[m])
            wk_m = w_pool.tile([P, HC * P], F8, tag="wk_m")
            nc.sync.dma_start(out=wk_m, in_=wk_d[m])
            return wq_m, wk_m

        def emit_q(m, wq_m):
            ps = ps_mm.tile([P, 512], F32, name="ps", tag="ps")
            for c2 in range(4):
                nc.tensor.matmul(
                    ps,
                    w_lhsT(wq_m, c2),
                    xt_rhs(c2, 0, 512),
                    start=(c2 == 0),
                    stop=(c2 == 3),
                    perf_mode=DR,
                )
            q_sb = kv_pool.tile([P, 512], F8, tag="q_sb")
            nc.vector.tensor_scalar_add(q_sb, ps, bqk[:, m : m + 1])
            qdr = kv_pool.tile([32, 2048], F8, tag="qdr")
            for h in range(2):
                for j in range(2):
                    nc.sync.dma_start(
                        out=qdr[:, 1024 * h + 512 * j : 1024 * h + 512 * (j + 1)],
                        in_=q_sb[64 * h + 32 * j : 64 * h + 32 * j + 32, :],
                    )
            return qdr

        def emit_k_block(m, wk_m, k_sb, n):
            ps = ps_mm.tile([P, 512], F32, name="ps", tag="ps")
            for c2 in range(4):
                nc.tensor.matmul(
                    ps,
                    w_lhsT(wk_m, c2),
                    xt_rhs(c2, n * 512, 512),
                    start=(c2 == 0),
                    stop=(c2 == 3),
                    perf_mode=DR,
                )
            nc.vector.tensor_scalar_add(
                k_sb[:, n * 512 : (n + 1) * 512], ps, bqk[:, 8 + m : 9 + m]
            )

        def emit_kdr_dma(k_sb, kdr, c0, c1):
            for h in range(2):
                for j in range(2):
                    nc.sync.dma_start(
                        out=kdr[:, 4096 * h + 2048 * j + c0 : 4096 * h + 2048 * j + c1],
                        in_=k_sb[64 * h + 32 * j : 64 * h + 32 * j + 32, c0:c1],
                    )

        def emit_qk_proj0():
            # pair 0 startup: only q + the first k block upfront; k blocks
            # 1-3 are interleaved into the t2 loop so their evacs (gated on
            # the late xT columns) don't head-of-line-block the V evacs on
            # the in-order DVE queue
            qdr = emit_q(0, wq0)
            k_sb = kv_pool.tile([P, S], F8, tag="k_sb")
            kdr = kv_pool.tile([32, 8192], F8, tag="kdr")
            emit_k_block(0, wk0, k_sb, 0)
            emit_kdr_dma(k_sb, kdr, 0, 512)
            # wv group 0 must beat the first ctx tiles, then the rest of xT
            wvt0 = wv_pool.tile([P, HC * 512], F8, name="wv0", tag="wv0")
            nc.sync.dma_start(out=wvt0, in_=wv_d[0])
            wv_g.append(wvt0)
            nc.sync.dma_start(out=xT_v[:, :, 512:1280], in_=xT_d[:, :, 512:1280])
            nc.sync.dma_start(out=xT_v[:, :, 1280:S], in_=xT_d[:, :, 1280:S])
            return qdr, kdr, k_sb

        def emit_v(g, t):
            ps = ps_mm.tile([P, 512], F32, name="vps", tag="ps")
            for c2 in range(4):
                nc.tensor.matmul(
                    ps,
                    xt_lhsT(c2, t),
                    wv_rhs(g, c2),
                    start=(c2 == 0),
                    stop=(c2 == 3),
                    perf_mode=DR,
                )
            nc.vector.tensor_scalar(
                out=va_dst(t, g),
                in0=_ap(ps, 0, [[P, 4], [64, 2], [1, 64]]),
                scalar1=em[:, t : t + 1],
                scalar2=None,
                op0=MULT,
            )

        # ---- main loop ----
        def emit_norm(m, ctx_ps):
            # normalize + fp8 ctxT (= 1024 * ctx); recips first so the DVE
            # chain pipelines across both heads
            rrs = []
            for h in range(2):
                rr = r_pool.tile([1, 512], F32R, tag="rr")
                nc.vector.reciprocal(rr, ctx_ps[h][64:65, :])
                rrs.append(rr)
            bcs = []
            for h in range(2):
                bc_ps = ps_mm.tile([64, 512], F32, name="bc_ps", tag="ps")
                nc.tensor.matmul(bc_ps, ones32_r, rrs[h], start=True, stop=True)
                bcs.append(bc_ps)
            rbs = []
            for h in range(2):
                rb = r_pool.tile([64, 512], F32, tag="rb")
                nc.vector.tensor_copy(rb, bcs[h])
                rbs.append(rb)
            for h in range(2):
                nc.vector.scalar_tensor_tensor(
                    out=ctxT_all[64 * h : 64 * h + 64, m * SQ : (m + 1) * SQ],
                    in0=ctx_ps[h][0:64, :],
                    scalar=WS,
                    in1=rbs[h],
                    op0=MULT,
                    op1=MULT,
                )

        wv_g = []
        qdr0, kdr0, k_sb0 = emit_qk_proj0()
        dr_cur = (qdr0, kdr0)
        wvt1 = wv_pool.tile([P, HC * 512], F8, name="wv1", tag="wv1")
        nc.sync.dma_start(out=wvt1, in_=wv_d[1])
        wv_g.append(wvt1)
        prev_norm = None  # (m, ctx_ps) awaiting normalization
        w_next = None
        k_next = None
        for m in range(NPAIR):
            qdr, kdr = dr_cur
            ctx_ps = [
                ps_ctx.tile([P, 512], F32, name=f"ctx{h}", tag="ctx_ps")
                for h in range(2)
            ]

            def emit_ctx(e_t2, e_pt):
                for h in range(2):
                    nc.tensor.matmul(
                        ctx_ps[h][0:96, :],
                        va_lhsT(e_t2, m, h),
                        pt_rhs(e_pt, h),
                        start=(e_t2 == 0),
                        stop=(e_t2 == 7),
                        perf_mode=DR,
                    )

            pend_ctx = []
            for t2 in range(8):
                pt = pt_pool.tile([P, 2048], F8, name="pt", tag="pt")
                for j in range(2):
                    t = 2 * t2 + j
                    s2 = ps_s.tile([P, 1024], F32, name="s2", tag="s2")
                    for h in range(2):
                        nc.tensor.matmul(
                            s2[:, 512 * h : 512 * (h + 1)],
                            kdr_lhsT(kdr, h, t),
                            qdr_rhs(qdr, h),
                            start=True,
                            stop=True,
                            perf_mode=DR,
                        )
                    if t in SCH_BY_PAIR[m]:
                        nc.vector.tensor_scalar(
                            out=pt[:, 1024 * j : 1024 * (j + 1)].bitcast(U8),
                            in0=s2,
                            scalar1=SCH_A,
                            scalar2=SCH_B,
                            op0=MULT,
                            op1=ADD,
                        )
                    else:
                        nc.scalar.activation(
                            pt[:, 1024 * j : 1024 * (j + 1)],
                            s2,
                            mybir.ActivationFunctionType.Exp,
                            scale=EXP_SCALE,
                        )
                # interleaved work while Act chews the exps; t2==2 keeps the
                # DVE norm chain away from the Schraudolph slots
                if t2 == 4 and prev_norm is not None:
                    emit_norm(*prev_norm)
                    prev_norm = None
                if m == 0:
                    emit_v(0, 2 * t2)
                    emit_v(0, 2 * t2 + 1)
                    if t2 in (0, 1, 2):
                        emit_k_block(0, wk0, k_sb0, t2 + 1)
                        emit_kdr_dma(
                            k_sb0, kdr0, 512 * (t2 + 1), 512 * (t2 + 2)
                        )
                elif m in V_SCHED and t2 in V_SCHED[m]:
                    for tv in V_SCHED[m][t2]:
                        emit_v(1, tv)
                if m < NPAIR - 1:
                    # next pair's proj, spread across t2 slots
                    if t2 == 1:
                        w_next = emit_w_dma(m + 1)
                    elif t2 == 2:
                        q_next = emit_q(m + 1, w_next[0])
                        k_next = (
                            kv_pool.tile([P, S], F8, tag="k_sb", name="k_sb"),
                            kv_pool.tile([32, 8192], F8, tag="kdr", name="kdr"),
                        )
                        dr_next = (q_next, k_next[1])
                    elif t2 in (3, 4, 5, 6):
                        emit_k_block(m + 1, w_next[1], k_next[0], t2 - 3)
                        if t2 == 6:
                            emit_kdr_dma(k_next[0], k_next[1], 0, 2048)
                if t2 == 5 and 2 <= m <= 6:
                    # deferred non-critical input DMAs, spread one pair apart
                    # so they never clump ahead of the next pair's weights
                    if m == 2:
                        xq = []
                    if m <= 5:
                        xqt = xq_pool.tile(
                            [P, H], F32, name=f"xq{m-2}", tag=f"xq{m-2}"
                        )
                        nc.sync.dma_start(
                            out=xqt, in_=xq_d[(m - 2) * P : (m - 1) * P, :]
                        )
                        xq.append(xqt)
                    if m == 3:
                        gamma_bc = consts.tile([P, H], F32, tag="gamma_bc")
                        nc.sync.dma_start(
                            out=gamma_bc, in_=gamma_d[:].partition_broadcast(P)
                        )
                    elif m == 4:
                        beta_bc = consts.tile([P, H], F32, tag="beta_bc")
                        nc.sync.dma_start(
                            out=beta_bc, in_=beta_d[:].partition_broadcast(P)
                        )
                    elif m == 6:
                        wo_sb = wv_pool.tile([P, HC * H], F8, name="wo", tag="wo")
                        nc.sync.dma_start(out=wo_sb, in_=wo_d[:, :])
                # defer ctx emission by 2 slots so a pending WAR on the
                # ctx accumulators (prev pair's norm) can't head-of-line
                # block the scores stream on the in-order PE
                pend_ctx.append((t2, pt))
                if len(pend_ctx) > 2:
                    emit_ctx(*pend_ctx.pop(0))
            for e in pend_ctx:
                emit_ctx(*e)
            prev_norm = (m, ctx_ps)
            dr_cur = dr_next
        emit_norm(*prev_norm)

        def wo_rhs(c2, nch):
            return _ap(wo_sb, 2 * c2 * H + nch * 512, [[H, 2], [1, 512]])

        # ---- output projection + residual + LayerNorm ----
        for st in range(SQ // P):
            h_sb = ln_pool.tile([P, H], F32, tag="h_sb")
            for nch in range(2):
                ps = ps_mm.tile([P, 512], F32, name="ops", tag="ps")
                for c2 in range(4):
                    nc.tensor.matmul(
                        ps,
                        ctxT_lhsT(c2, st),
                        wo_rhs(c2, nch),
                        start=(c2 == 0),
                        stop=(c2 == 3),
                        perf_mode=DR,
                    )
                h0 = ln_pool.tile([P, 512], F32, tag="h0")
                nc.scalar.mul(h0, ps, OUT_SCALE)
                nc.vector.tensor_add(
                    h_sb[:, nch * 512 : (nch + 1) * 512],
                    h0,
                    xq[st][:, nch * 512 : (nch + 1) * 512],
                )
            stats = ln_pool.tile([P, 2, 6], F32, tag="stats")
            for gg in range(2):
                nc.vector.bn_stats(
                    out=stats[:, gg, :], in_=h_sb[:, gg * 512 : (gg + 1) * 512]
                )
            mv = ln_pool.tile([P, 2], F32, tag="mv")
            nc.vector.bn_aggr(out=mv, in_=stats)
            sd = ln_pool.tile([P, 1], F32, tag="sd")
            nc.scalar.activation(
                sd, mv[:, 1:2], mybir.ActivationFunctionType.Sqrt, bias=eps_t
            )
            rs = ln_pool.tile([P, 1], F32, tag="rs")
            nc.vector.reciprocal(rs, sd)
            t1 = ln_pool.tile([P, H], F32, tag="t1")
            nc.vector.scalar_tensor_tensor(
                out=t1, in0=h_sb, scalar=mv[:, 0:1], in1=gamma_bc, op0=SUB, op1=MULT
            )
            ob = ln_pool.tile([P, H], F32, tag="ob")
            if st >= 2:
                # final stage alternates DVE/Pool so neither serializes the tail
                nc.vector.scalar_tensor_tensor(
                    out=ob, in0=t1, scalar=rs, in1=beta_bc, op0=MULT, op1=ADD
                )
            else:
                t2_ = ln_pool.tile([P, H], F32, tag="t2_")
                nc.gpsimd.tensor_scalar(
                    out=t2_, in0=t1, scalar1=rs, scalar2=None, op0=MULT
                )
                nc.gpsimd.tensor_tensor(out=ob, in0=t2_, in1=beta_bc, op=ADD)
            nc.sync.dma_start(out=out_d[st * P : (st + 1) * P, :], in_=ob)

        for _pool in (ps_ctx, ps_s, ps_mm, ln_pool, r_pool, pt_pool, kv_pool,
                      w_pool, xq_pool, ctxT_pool, wv_pool, va_pool, xT_pool,
                      consts):
            _pool.release()

    _split_excess_waits(nc)
    return nc


_NC = None


def _get_nc():
    global _NC
    if _NC is None:
        _NC = build_nc()
    return _NC


def _in_maps(hidden_states, attention_mask, Wq, bq, Wk, bk, Wv, bv, Wo, bo, gamma, beta):
    f8 = ml_dtypes.float8_e4m3
    hs = np.asarray(hidden_states, dtype=np.float32)
    am = np.asarray(attention_mask, dtype=np.float32).reshape(2, S)
    Wo_f = np.asarray(Wo, dtype=np.float32)

    def pair_w(w):
        w = np.asarray(w, dtype=np.float32) * WS
        return np.ascontiguousarray(
            w.reshape(HC, P, NPAIR, P).transpose(2, 1, 0, 3).reshape(NPAIR, P, H)
        ).astype(f8)

    wq_t, wk_t = pair_w(Wq), pair_w(Wk)
    wv_t = np.ascontiguousarray(
        (np.asarray(Wv, dtype=np.float32) * WS)
        .reshape(HC, P, 2, 512)
        .transpose(2, 1, 0, 3)
        .reshape(2, P, HC * 512)
    ).astype(f8)
    wo_t = np.ascontiguousarray(
        (Wo_f * WS).reshape(HC, P, H).transpose(1, 0, 2).reshape(P, HC * H)
    ).astype(f8)
    bqk = np.ascontiguousarray(
        np.concatenate(
            [
                (np.asarray(b, dtype=np.float32) * WS).reshape(NPAIR, P).T
                for b in (bq, bk)
            ],
            axis=1,
        )
    )
    g_c = np.ascontiguousarray(np.asarray(gamma, dtype=np.float32))
    be_c = np.ascontiguousarray(np.asarray(beta, dtype=np.float32))
    # residual folds: x + bo + bv @ Wo
    res_c = (
        np.asarray(bo, dtype=np.float32)
        + np.asarray(bv, dtype=np.float32) @ Wo_f
    )

    maps = []
    for core in range(8):
        b, j = core // 4, core % 4
        # roll the sequence so this core's query block is always cols [0, 512);
        # attention sums over all keys, so key order is irrelevant as long as
        # the multiplicative mask em is rolled identically.
        xs = np.roll(hs[b], -j * SQ, axis=0)
        ms = np.roll(am[b], -j * SQ)
        xT = np.ascontiguousarray(
            xs.T.reshape(HC, P, S).transpose(1, 0, 2)
        ).astype(f8)
        maps.append(
            {
                "xT": xT,
                "xq": np.ascontiguousarray(xs[0:SQ, :] + res_c),
                "wq": wq_t,
                "wk": wk_t,
                "wv": wv_t,
                "wo": wo_t,
                "bqk": bqk,
                "gamma": g_c,
                "beta": be_c,
                "em": np.ascontiguousarray(np.exp(ms).reshape(NKT, P).T),
            }
        )
    return maps


def run(trace=False, **inputs):
    nc = _get_nc()
    maps = _in_maps(**inputs)
    res = run_bass_kernel_spmd(nc, maps, core_ids=list(range(8)), trace=trace)
    out = np.empty((2, S, H), dtype=np.float32)
    for core in range(8):
        b, j = core // 4, core % 4
        out[b, j * SQ : (j + 1) * SQ, :] = res.results[core]["out"]
    return out, res


def kernel(**inputs):
    out, _ = run(trace=False, **inputs)
    return out


# revision 7
# speedup vs baseline: 1.0640x; 1.0287x over previous
"""BertAttention Trainium2 kernel v2 (8 NeuronCores, SPMD, no collectives).

Sharding: DP over batch (2) x sequence-parallel over 512-row query blocks (4).
All heavy matmuls run fp8e4 DoubleRow (2 contraction subtiles per instruction,
0.5 cyc/output-column):

  - host sends xT pre-transposed fp8 (sequence rolled so this core's query
    block is cols [0,512)), weights pre-tiled fp8 scaled by 32 (keeps
    W ~N(0,0.02^2) out of the fp8 subnormal range), biases f32.
  - attention_mask folded on host: em = exp(mask) scales the vaug rows
    (data cols and the denominator ones-col), so exp needs no bias;
    bv and bo fold into the residual (xq += bo + bv @ Wo).
  - Q/K proj per head-pair in [d, seq] orientation; evac adds 32*bias, casts
    fp8 (DVE). scores contract d=64 as two 32-partition DoubleRow subtiles
    via DMA-rearranged kdr/qdr. V computed in natural [sk, d] orientation,
    4 head-pairs per PSUM; evac scales by em[sk], casts fp8 into vaug
    [128, j, 96] tiles (col 64 = em, cols 65..95 zero pad - DoubleRow wants
    M % 32 == 0). ctx PSUM row 64 accumulates the softmax denominator.
  - softmax exp on Act engine; a subset of tiles is offloaded to DVE via a
    Schraudolph-style trick: u8 = round(8*log2(pt)+56) bitcast to fp8e4.
  - ctx /= den via fast reciprocal + ones(1.0) K=1 matmul broadcast; the
    fp8 cast multiplies by 32 (scalar_tensor_tensor) so ctxT = 1024*ctx.
  - out-proj evac: Act scale-copy 2^-15 + DVE residual add; LayerNorm via
    bn_stats/aggr, Act Rsqrt, fused DVE stt + Pool tensor_scalar/add tail.
"""

import numpy as np
import ml_dtypes

import bass_rust as _br
import concourse.bass as bass
import concourse.tile as tile
from concourse import mybir
from concourse.bass_utils import run_bass_kernel_spmd

F32 = mybir.dt.float32
F32R = mybir.dt.float32r
F8 = mybir.dt.float8e4
U8 = mybir.dt.uint8
DR = mybir.MatmulPerfMode.DoubleRow
ADD = mybir.AluOpType.add
MULT = mybir.AluOpType.mult
SUB = mybir.AluOpType.subtract

S = 2048
H = 1024
P = 128
SQ = 512          # query rows per core
NKT = S // P      # 16 sk tiles
HC = H // P       # 8 h-chunks
NPAIR = 8         # head pairs
WS = 32.0         # fp8 weight scale
EPS = 1e-12
EXP_SCALE = 0.125 / (WS * WS)     # 2^-13
OUT_SCALE = 1.0 / (WS * WS * WS)  # 2^-15
# Schraudolph-on-DVE offload: these sk-tiles' exps run on DVE as
# u8 = s * SCH_A + SCH_B bitcast fp8e4 (exp2 piecewise-linear approx).
# Per-pair sets sized to balance DVE load (pairs 0-3 also carry V evacs).
SCH_BY_PAIR = (
    (),
    (7, 10, 13),
    (7, 10, 13),
    (7, 10, 13),
    (3, 7, 9, 11, 14),
    (3, 7, 9, 11, 14),
    (3, 7, 9, 11, 14),
    (2, 5, 8, 11, 14),
)
SCH_A = 8 * 1.4426950408889634 * EXP_SCALE
SCH_B = 56.0
# V-projection emission schedule: group 0 just-in-time inside pair 0;
# group 1 spread over pairs 1-3 (list of t per (pair, t2 slot)).
V_SCHED = {
    1: {0: (0, 1), 1: (2, 3), 2: (4, 5)},
    2: {0: (6, 7), 1: (8, 9), 2: (10,)},
    3: {0: (11, 12), 1: (13, 14), 2: (15,)},
}

_wait_ctr = [0]


def _split_excess_waits(nc, limit=1):
    """walrus in this container rejects >1-2 sem waits on several opcode
    structs; move excess waits onto same-engine NoOps inserted just before."""
    for f in nc.m.functions:
        for bb in f.blocks:
            insts = bb.instructions
            out = []
            dirty = False
            for inst in insts:
                si = inst.sync_info
                waits = list(si.on_wait) if si and si.on_wait else []
                if len(waits) > limit and inst.engine != mybir.EngineType.Unassigned:
                    for i in range(0, len(waits) - limit, limit):
                        _wait_ctr[0] += 1
                        nop = _br.InstNoOp(
                            name=f"I-waitsplit-{_wait_ctr[0]}", ins=[], outs=[]
                        )
                        nop.engine = inst.engine
                        nop.sync_info = mybir.SyncInfo(
                            on_wait=waits[i : i + limit], on_update=[]
                        )
                        out.append(nop)
                    si.on_wait = waits[len(waits) - limit :]
                    dirty = True
                out.append(inst)
            if dirty:
                bb.instructions = out
    return nc


def _ap(t, off, dims):
    """Strided AP over a tile's partition range: dims = [[stride, count], ...]."""
    return bass.AP(
        tensor=t.tensor,
        offset=t.offset + off,
        ap=[list(t.ap[0])] + [list(d) for d in dims],
    )


# vaug layout per partition: [t2(8), j(2), m(8), h(2), c(96)]
VA_C = 96
VA_H = VA_C            # stride of h
VA_M = 2 * VA_C        # 192
VA_J = 8 * VA_M        # 1536
VA_T2 = 2 * VA_J       # 3072
VA_TOTAL = 8 * VA_T2   # 24576


def build_nc():
    nc = bass.Bass()

    xT_d = nc.dram_tensor("xT", [P, HC, S], F8, kind="ExternalInput")
    xq_d = nc.dram_tensor("xq", [SQ, H], F32, kind="ExternalInput")  # +bo+bv@Wo
    wq_d = nc.dram_tensor("wq", [NPAIR, P, HC * P], F8, kind="ExternalInput")
    wk_d = nc.dram_tensor("wk", [NPAIR, P, HC * P], F8, kind="ExternalInput")
    wv_d = nc.dram_tensor("wv", [2, P, HC * 512], F8, kind="ExternalInput")
    wo_d = nc.dram_tensor("wo", [P, HC * H], F8, kind="ExternalInput")
    bqk_d = nc.dram_tensor("bqk", [P, 16], F32, kind="ExternalInput")  # 32*(bq|bk)
    gamma_d = nc.dram_tensor("gamma", [H], F32, kind="ExternalInput")
    beta_d = nc.dram_tensor("beta", [H], F32, kind="ExternalInput")
    em_d = nc.dram_tensor("em", [P, NKT], F32, kind="ExternalInput")  # exp(mask)
    out_d = nc.dram_tensor("out", [SQ, H], F32, kind="ExternalOutput")

    with tile.TileContext(nc) as tc, nc.allow_low_precision(
        reason="fp8 DoubleRow matmuls; accumulation stays fp32 in PSUM"
    ):
        consts = tc.alloc_tile_pool(name="consts", bufs=1)
        xT_pool = tc.alloc_tile_pool(name="xT", bufs=1)
        va_pool = tc.alloc_tile_pool(name="va", bufs=1)
        wv_pool = tc.alloc_tile_pool(name="wv", bufs=1)
        ctxT_pool = tc.alloc_tile_pool(name="ctxT", bufs=1)
        xq_pool = tc.alloc_tile_pool(name="xq", bufs=1)
        w_pool = tc.alloc_tile_pool(name="w", bufs=3)
        kv_pool = tc.alloc_tile_pool(name="kv", bufs=4)
        pt_pool = tc.alloc_tile_pool(name="pt", bufs=6)
        r_pool = tc.alloc_tile_pool(name="r", bufs=4)
        ln_pool = tc.alloc_tile_pool(name="ln", bufs=2)
        ps_mm = tc.alloc_tile_pool(name="ps_mm", bufs=2, space="PSUM")
        ps_s = tc.alloc_tile_pool(name="ps_s", bufs=2, space="PSUM")
        ps_ctx = tc.alloc_tile_pool(name="ps_ctx", bufs=2, space="PSUM")

        # ---- critical-path DMAs first (HWDGE is ~620ns PER DMA op and the
        # transfers mostly serialize, so few, large, well-ordered DMAs
        # matter): xT query cols, then wq0/wk0 ----
        xT_all = xT_pool.tile([P, HC * S], F8, name="xT_all", tag="xT_all")
        xT_v = xT_all.rearrange("p (c s) -> p c s", c=HC)
        # query-block columns (0..512) of every chunk in one strided DMA:
        # unblocks the q projection and the first k block early
        nc.sync.dma_start(out=xT_v[:, :, 0:512], in_=xT_d[:, :, 0:512])
        wq0 = w_pool.tile([P, HC * P], F8, tag="wq_m")
        nc.sync.dma_start(out=wq0, in_=wq_d[0])
        wk0 = w_pool.tile([P, HC * P], F8, tag="wk_m")
        nc.sync.dma_start(out=wk0, in_=wk_d[0])
        bqk = consts.tile([P, 16], F32, tag="bqk")
        nc.sync.dma_start(out=bqk, in_=bqk_d[:, :])
        em = consts.tile([P, NKT], F32, tag="em")
        nc.sync.dma_start(out=em, in_=em_d[:, :])

        # ---- small consts / VA init (Pool) ----
        eps_t = consts.tile([P, 1], F32, tag="eps")
        nc.vector.memset(eps_t, EPS)
        ones32 = consts.tile([1, 64], F32, tag="ones32")
        nc.vector.memset(ones32, 1.0)
        ones32_r = ones32.bitcast(F32R)
        zeros16 = consts.tile([P, 16], F32, tag="zeros16")
        nc.gpsimd.memset(zeros16, 0.0)

        VA = va_pool.tile([P, VA_TOTAL], F8, name="VA", tag="VA")
        for t2 in range(8):
            for j in range(2):
                base = VA_T2 * t2 + VA_J * j
                # zero the pad block (cols 64..95)
                nc.gpsimd.memset(
                    _ap(VA, base + 64, [[VA_M, 8], [VA_H, 2], [1, 32]]), 0.0
                )
        for t in range(NKT):
            # denominator col (64) = em[:, t] per (m, h)
            base = VA_T2 * (t // 2) + VA_J * (t % 2) + 64
            nc.gpsimd.tensor_scalar(
                out=_ap(VA, base, [[VA_M, 8], [VA_H, 2]]),
                in0=_ap(zeros16, 0, [[2, 8], [1, 2]]),
                scalar1=em[:, t : t + 1],
                scalar2=None,
                op0=ADD,
            )

        ctxT_all = ctxT_pool.tile([P, NPAIR * SQ], F8, name="ctxT", tag="ctxT")

        # ---- AP helpers ----
        def xt_rhs(c2, s0, ns):
            return _ap(xT_all, 2 * c2 * S + s0, [[S, 2], [1, ns]])

        def xt_lhsT(c2, t):
            return _ap(xT_all, 2 * c2 * S + t * P, [[S, 2], [1, P]])

        def w_lhsT(w, c2):
            return _ap(w, 2 * c2 * P, [[P, 2], [1, P]])

        def wv_rhs(g, c2):
            return _ap(wv_g[g], 2 * c2 * 512, [[512, 2], [1, 512]])

        def va_lhsT(t2, m, h):
            return _ap(VA, VA_T2 * t2 + VA_M * m + VA_H * h, [[VA_J, 2], [1, VA_C]])

        def va_dst(t, g):
            return _ap(
                VA,
                VA_T2 * (t // 2) + VA_J * (t % 2) + VA_M * (4 * g),
                [[VA_M, 4], [VA_H, 2], [1, 64]],
            )

        def kdr_lhsT(kdr, h, t):
            return _ap(kdr, 4096 * h + t * P, [[2048, 2], [1, P]])

        def qdr_rhs(qdr, h):
            return _ap(qdr, 1024 * h, [[512, 2], [1, 512]])

        def pt_rhs(pt, h):
            return _ap(pt, 512 * h, [[1024, 2], [1, 512]])

        def ctxT_lhsT(c2, st):
            return _ap(ctxT_all, 2 * c2 * SQ + st * P, [[SQ, 2], [1, P]])

        # ---- per-pair Q/K projection + DoubleRow rearrangements ----
        def emit_w_dma(m):
            wq_m = w_pool.tile([P, HC * P], F8, tag="wq_m")
            nc.sync.dma_start(out=wq_m, in_=wq_d[m])
            wk_m = w_pool.tile([P, HC * P], F8, tag="wk_m")
            nc.sync.dma_start(out=wk_m, in_=wk_d[m])
            return wq_m, wk_m

        def emit_q(m, wq_m):
            ps = ps_mm.tile([P, 512], F32, name="ps", tag="ps")
            for c2 in range(4):
                nc.tensor.matmul(
                    ps,
                    w_lhsT(wq_m, c2),
                    xt_rhs(c2, 0, 512),
                    start=(c2 == 0),
                    stop=(c2 == 3),
                    perf_mode=DR,
                )
            q_sb = kv_pool.tile([P, 512], F8, tag="q_sb")
            nc.vector.tensor_scalar_add(q_sb, ps, bqk[:, m : m + 1])
            qdr = kv_pool.tile([32, 2048], F8, tag="qdr")
            for h in range(2):
                for j in range(2):
                    nc.sync.dma_start(
                        out=qdr[:, 1024 * h + 512 * j : 1024 * h + 512 * (j + 1)],
                        in_=q_sb[64 * h + 32 * j : 64 * h + 32 * j + 32, :],
                    )
            return qdr, q_sb

        def emit_k_block(m, wk_m, k_sb, n):
            ps = ps_mm.tile([P, 512], F32, name="ps", tag="ps")
            for c2 in range(4):
                nc.tensor.matmul(
                    ps,
                    w_lhsT(wk_m, c2),
                    xt_rhs(c2, n * 512, 512),
                    start=(c2 == 0),
                    stop=(c2 == 3),
                    perf_mode=DR,
                )
            nc.vector.tensor_scalar_add(
                k_sb[:, n * 512 : (n + 1) * 512], ps, bqk[:, 8 + m : 9 + m]
            )

        def emit_kdr_dma(k_sb, kdr, c0, c1):
            for h in range(2):
                for j in range(2):
                    nc.sync.dma_start(
                        out=kdr[:, 4096 * h + 2048 * j + c0 : 4096 * h + 2048 * j + c1],
                        in_=k_sb[64 * h + 32 * j : 64 * h + 32 * j + 32, c0:c1],
                    )

        def emit_qk_proj0():
            # pair 0 startup: only q + the first k block upfront; k blocks
            # 1-3 are interleaved into the t2 loop so their evacs (gated on
            # the late xT columns) don't head-of-line-block the V evacs on
            # the in-order DVE queue. No n0 kdr DMA: pair 0's first four
            # score tiles run non-DoubleRow straight from k_sb/q_sb, so the
            # first exp never waits on the 32-partition rearrange.
            qdr, q_sb0_ = emit_q(0, wq0)
            k_sb = kv_pool.tile([P, S], F8, tag="k_sb")
            kdr = kv_pool.tile([32, 8192], F8, tag="kdr")
            emit_k_block(0, wk0, k_sb, 0)
            # wv group 0 must beat the first ctx tiles, then the rest of xT
            wvt0 = wv_pool.tile([P, HC * 512], F8, name="wv0", tag="wv0")
            nc.sync.dma_start(out=wvt0, in_=wv_d[0])
            wv_g.append(wvt0)
            nc.sync.dma_start(out=xT_v[:, :, 512:1280], in_=xT_d[:, :, 512:1280])
            nc.sync.dma_start(out=xT_v[:, :, 1280:S], in_=xT_d[:, :, 1280:S])
            return qdr, kdr, k_sb, q_sb0_

        def emit_v(g, t):
            ps = ps_mm.tile([P, 512], F32, name="vps", tag="ps")
            for c2 in range(4):
                nc.tensor.matmul(
                    ps,
                    xt_lhsT(c2, t),
                    wv_rhs(g, c2),
                    start=(c2 == 0),
                    stop=(c2 == 3),
                    perf_mode=DR,
                )
            nc.vector.tensor_scalar(
                out=va_dst(t, g),
                in0=_ap(ps, 0, [[P, 4], [64, 2], [1, 64]]),
                scalar1=em[:, t : t + 1],
                scalar2=None,
                op0=MULT,
            )

        # ---- main loop ----
        def emit_norm(m, ctx_ps):
            # normalize + fp8 ctxT (= 1024 * ctx); recips first so the DVE
            # chain pipelines across both heads
            rrs = []
            for h in range(2):
                rr = r_pool.tile([1, 512], F32R, tag="rr")
                nc.vector.reciprocal(rr, ctx_ps[h][64:65, :])
                rrs.append(rr)
            bcs = []
            for h in range(2):
                bc_ps = ps_mm.tile([64, 512], F32, name="bc_ps", tag="ps")
                nc.tensor.matmul(bc_ps, ones32_r, rrs[h], start=True, stop=True)
                bcs.append(bc_ps)
            rbs = []
            for h in range(2):
                rb = r_pool.tile([64, 512], F32, tag="rb")
                nc.vector.tensor_copy(rb, bcs[h])
                rbs.append(rb)
            for h in range(2):
                nc.vector.scalar_tensor_tensor(
                    out=ctxT_all[64 * h : 64 * h + 64, m * SQ : (m + 1) * SQ],
                    in0=ctx_ps[h][0:64, :],
                    scalar=WS,
                    in1=rbs[h],
                    op0=MULT,
                    op1=MULT,
                )

        wv_g = []
        qdr0, kdr0, k_sb0, q_sb0 = emit_qk_proj0()
        dr_cur = (qdr0, kdr0)
        wvt1 = wv_pool.tile([P, HC * 512], F8, name="wv1", tag="wv1")
        nc.sync.dma_start(out=wvt1, in_=wv_d[1])
        wv_g.append(wvt1)
        prev_norm = None  # (m, ctx_ps) awaiting normalization
        w_next = None
        k_next = None
        for m in range(NPAIR):
            qdr, kdr = dr_cur
            ctx_ps = [
                ps_ctx.tile([P, 512], F32, name=f"ctx{h}", tag="ctx_ps")
                for h in range(2)
            ]

            def emit_ctx(e_t2, e_pt):
                for h in range(2):
                    nc.tensor.matmul(
                        ctx_ps[h][0:96, :],
                        va_lhsT(e_t2, m, h),
                        pt_rhs(e_pt, h),
                        start=(e_t2 == 0),
                        stop=(e_t2 == 7),
                        perf_mode=DR,
                    )

            pend_ctx = []
            for t2 in range(8):
                pt = pt_pool.tile([P, 2048], F8, name="pt", tag="pt")
                for j in range(2):
                    t = 2 * t2 + j
                    s2 = ps_s.tile([P, 1024], F32, name="s2", tag="s2")
                    for h in range(2):
                        if m == 0 and t < 4:
                            nc.tensor.matmul(
                                s2[:, 512 * h : 512 * (h + 1)],
                                k_sb0[64 * h : 64 * h + 64, t * P : (t + 1) * P],
                                q_sb0[64 * h : 64 * h + 64, :],
                                start=True,
                                stop=True,
                            )
                        else:
                            nc.tensor.matmul(
                                s2[:, 512 * h : 512 * (h + 1)],
                                kdr_lhsT(kdr, h, t),
                                qdr_rhs(qdr, h),
                                start=True,
                                stop=True,
                                perf_mode=DR,
                            )
                    if t in SCH_BY_PAIR[m]:
                        nc.vector.tensor_scalar(
                            out=pt[:, 1024 * j : 1024 * (j + 1)].bitcast(U8),
                            in0=s2,
                            scalar1=SCH_A,
                            scalar2=SCH_B,
                            op0=MULT,
                            op1=ADD,
                        )
                    else:
                        nc.scalar.activation(
                            pt[:, 1024 * j : 1024 * (j + 1)],
                            s2,
                            mybir.ActivationFunctionType.Exp,
                            scale=EXP_SCALE,
                        )
                # interleaved work while Act chews the exps; t2==2 keeps the
                # DVE norm chain away from the Schraudolph slots
                if t2 == 3 and prev_norm is not None:
                    emit_norm(*prev_norm)
                    prev_norm = None
                if m == 0:
                    emit_v(0, 2 * t2)
                    emit_v(0, 2 * t2 + 1)
                    if t2 in (0, 1, 2):
                        emit_k_block(0, wk0, k_sb0, t2 + 1)
                        emit_kdr_dma(
                            k_sb0, kdr0, 512 * (t2 + 1), 512 * (t2 + 2)
                        )
                elif m in V_SCHED and t2 in V_SCHED[m]:
                    for tv in V_SCHED[m][t2]:
                        emit_v(1, tv)
                if m < NPAIR - 1:
                    # next pair's proj, spread across t2 slots (early, with
                    # per-block kdr DMAs, so pair m+1's first scores never
                    # wait on the rearrange)
                    if t2 == 0:
                        w_next = emit_w_dma(m + 1)
                    elif t2 == 1:
                        q_next, _ = emit_q(m + 1, w_next[0])
                        k_next = (
                            kv_pool.tile([P, S], F8, tag="k_sb", name="k_sb"),
                            kv_pool.tile([32, 8192], F8, tag="kdr", name="kdr"),
                        )
                        dr_next = (q_next, k_next[1])
                    elif t2 in (2, 3, 4, 5):
                        emit_k_block(m + 1, w_next[1], k_next[0], t2 - 2)
                        emit_kdr_dma(
                            k_next[0], k_next[1], 512 * (t2 - 2), 512 * (t2 - 1)
                        )
                if t2 == 5 and 2 <= m <= 6:
                    # deferred non-critical input DMAs, spread one pair apart
                    # so they never clump ahead of the next pair's weights
                    if m == 2:
                        xq = []
                    if m <= 5:
                        xqt = xq_pool.tile(
                            [P, H], F32, name=f"xq{m-2}", tag=f"xq{m-2}"
                        )
                        nc.sync.dma_start(
                            out=xqt, in_=xq_d[(m - 2) * P : (m - 1) * P, :]
                        )
                        xq.append(xqt)
                    if m == 3:
                        gamma_bc = consts.tile([P, H], F32, tag="gamma_bc")
                        nc.sync.dma_start(
                            out=gamma_bc, in_=gamma_d[:].partition_broadcast(P)
                        )
                    elif m == 4:
                        beta_bc = consts.tile([P, H], F32, tag="beta_bc")
                        nc.sync.dma_start(
                            out=beta_bc, in_=beta_d[:].partition_broadcast(P)
                        )
                    elif m == 6:
                        wo_sb = wv_pool.tile([P, HC * H], F8, name="wo", tag="wo")
                        nc.sync.dma_start(out=wo_sb, in_=wo_d[:, :])
                # defer ctx emission by 2 slots so a pending WAR on the
                # ctx accumulators (prev pair's norm) can't head-of-line
                # block the scores stream on the in-order PE
                pend_ctx.append((t2, pt))
                if len(pend_ctx) > 2:
                    emit_ctx(*pend_ctx.pop(0))
            for e in pend_ctx:
                emit_ctx(*e)
            prev_norm = (m, ctx_ps)
            dr_cur = dr_next
        emit_norm(*prev_norm)

        def wo_rhs(c2, nch):
            return _ap(wo_sb, 2 * c2 * H + nch * 512, [[H, 2], [1, 512]])

        # ---- output projection + residual + LayerNorm ----
        for st in range(SQ // P):
            h_sb = ln_pool.tile([P, H], F32, tag="h_sb")
            for nch in range(2):
                ps = ps_mm.tile([P, 512], F32, name="ops", tag="ps")
                for c2 in range(4):
                    nc.tensor.matmul(
                        ps,
                        ctxT_lhsT(c2, st),
                        wo_rhs(c2, nch),
                        start=(c2 == 0),
                        stop=(c2 == 3),
                        perf_mode=DR,
                    )
                h0 = ln_pool.tile([P, 512], F32, tag="h0")
                nc.scalar.mul(h0, ps, OUT_SCALE)
                nc.vector.tensor_add(
                    h_sb[:, nch * 512 : (nch + 1) * 512],
                    h0,
                    xq[st][:, nch * 512 : (nch + 1) * 512],
                )
            stats = ln_pool.tile([P, 2, 6], F32, tag="stats")
            for gg in range(2):
                nc.vector.bn_stats(
                    out=stats[:, gg, :], in_=h_sb[:, gg * 512 : (gg + 1) * 512]
                )
            mv = ln_pool.tile([P, 2], F32, tag="mv")
            nc.vector.bn_aggr(out=mv, in_=stats)
            sd = ln_pool.tile([P, 1], F32, tag="sd")
            nc.scalar.activation(
                sd, mv[:, 1:2], mybir.ActivationFunctionType.Sqrt, bias=eps_t
            )
            rs = ln_pool.tile([P, 1], F32, tag="rs")
            nc.vector.reciprocal(rs, sd)
            t1 = ln_pool.tile([P, H], F32, tag="t1")
            nc.vector.scalar_tensor_tensor(
                out=t1, in0=h_sb, scalar=mv[:, 0:1], in1=gamma_bc, op0=SUB, op1=MULT
            )
            ob = ln_pool.tile([P, H], F32, tag="ob")
            if st >= 2:
                # final stage alternates DVE/Pool so neither serializes the tail
                nc.vector.scalar_tensor_tensor(
                    out=ob, in0=t1, scalar=rs, in1=beta_bc, op0=MULT, op1=ADD
                )
            else:
                t2_ = ln_pool.tile([P, H], F32, tag="t2_")
                nc.gpsimd.tensor_scalar(
                    out=t2_, in0=t1, scalar1=rs, scalar2=None, op0=MULT
                )
                nc.gpsimd.tensor_tensor(out=ob, in0=t2_, in1=beta_bc, op=ADD)
            nc.sync.dma_start(out=out_d[st * P : (st + 1) * P, :], in_=ob)

        for _pool in (ps_ctx, ps_s, ps_mm, ln_pool, r_pool, pt_pool, kv_pool,
                      w_pool, xq_pool, ctxT_pool, wv_pool, va_pool, xT_pool,
                      consts):
            _pool.release()

    _split_excess_waits(nc)
    return nc


_NC = None


def _get_nc():
    global _NC
    if _NC is None:
        _NC = build_nc()
    return _NC


def _in_maps(hidden_states, attention_mask, Wq, bq, Wk, bk, Wv, bv, Wo, bo, gamma, beta):
    f8 = ml_dtypes.float8_e4m3
    hs = np.asarray(hidden_states, dtype=np.float32)
    am = np.asarray(attention_mask, dtype=np.float32).reshape(2, S)
    Wo_f = np.asarray(Wo, dtype=np.float32)

    def pair_w(w):
        w = np.asarray(w, dtype=np.float32) * WS
        return np.ascontiguousarray(
            w.reshape(HC, P, NPAIR, P).transpose(2, 1, 0, 3).reshape(NPAIR, P, H)
        ).astype(f8)

    wq_t, wk_t = pair_w(Wq), pair_w(Wk)
    wv_t = np.ascontiguousarray(
        (np.asarray(Wv, dtype=np.float32) * WS)
        .reshape(HC, P, 2, 512)
        .transpose(2, 1, 0, 3)
        .reshape(2, P, HC * 512)
    ).astype(f8)
    wo_t = np.ascontiguousarray(
        (Wo_f * WS).reshape(HC, P, H).transpose(1, 0, 2).reshape(P, HC * H)
    ).astype(f8)
    bqk = np.ascontiguousarray(
        np.concatenate(
            [
                (np.asarray(b, dtype=np.float32) * WS).reshape(NPAIR, P).T
                for b in (bq, bk)
            ],
            axis=1,
        )
    )
    g_c = np.ascontiguousarray(np.asarray(gamma, dtype=np.float32))
    be_c = np.ascontiguousarray(np.asarray(beta, dtype=np.float32))
    # residual folds: x + bo + bv @ Wo
    res_c = (
        np.asarray(bo, dtype=np.float32)
        + np.asarray(bv, dtype=np.float32) @ Wo_f
    )

    maps = []
    for core in range(8):
        b, j = core // 4, core % 4
        # roll the sequence so this core's query block is always cols [0, 512);
        # attention sums over all keys, so key order is irrelevant as long as
        # the multiplicative mask em is rolled identically.
        xs = np.roll(hs[b], -j * SQ, axis=0)
        ms = np.roll(am[b], -j * SQ)
        xT = np.ascontiguousarray(
            xs.T.reshape(HC, P, S).transpose(1, 0, 2)
        ).astype(f8)
        maps.append(
            {
                "xT": xT,
                "xq": np.ascontiguousarray(xs[0:SQ, :] + res_c),
                "wq": wq_t,
                "wk": wk_t,
                "wv": wv_t,
                "wo": wo_t,
                "bqk": bqk,
                "gamma": g_c,
                "beta": be_c,
                "em": np.ascontiguousarray(np.exp(ms).reshape(NKT, P).T),
            }
        )
    return maps


def run(trace=False, **inputs):
    nc = _get_nc()
    maps = _in_maps(**inputs)
    res = run_bass_kernel_spmd(nc, maps, core_ids=list(range(8)), trace=trace)
    out = np.empty((2, S, H), dtype=np.float32)
    for core in range(8):
        b, j = core // 4, core % 4
        out[b, j * SQ : (j + 1) * SQ, :] = res.results[core]["out"]
    return out, res


def kernel(**inputs):
    out, _ = run(trace=False, **inputs)
    return out
